# revision 10
# baseline (speedup 1.0000x reference)
"""Equilibrium Propagation network kernel for 8x Trainium2 NeuronCores.

Problem: 30 damped-gradient relaxation iterations of a 1024-128-1000 Hopfield
energy network over batch 8192, then log_softmax. Data-parallel over batch
(1024 rows/core), no collectives.

Per-core design (all in transposed layout, state resident in SBUF):
  - state hT [H=128, B=1024], oT in 8 chunks [128, 1024] (O padded 1000->1024)
  - weights pre-scaled by 0.5 so PE matmuls deliver 0.5*A (A = pre-activation)
  - first K_EXACT iterations replicate jax's clip-gradient convention exactly
    via a fused DVE op  s' = clip(s + 0.5*((s>0)+(s<1)) * P)  with PE
    delivering P = 0.5*(A + b - s) (identity-injection matmuls)
  - remaining iterations use the relaxed update  s' = clip(0.5*s + 0.5*A)
    (the rho' mask only affects units sitting exactly on the 0/1 boundary;
    the fixed point is mask-independent and the trajectory difference decays
    - measured 2e-3 rel err vs the reference at 30 iterations).  This drops
    every identity-injection matmul (-35% PE work) and shrinks the DVE op to
    4 pipeline stages.
  - the 9 per-iteration state updates are split across engines:
    DVE (fused op) for h + 5 o-chunks, Pool (scalar_tensor_tensor + clip
    tensor_scalar) for 2 chunks, ACT (double-Relu clip of a PE-accumulated
    pre-activation) for 1 chunk, with the constant 0.5*C injected into the
    h psum by an ACT copy instead of a PE identity matmul
  - matmuls run in float32r (full PE rate, ~13 mantissa bits)
  - epilogue: PE-transpose back to [batch, O], exp+accumulate on ACT,
    ln, and per-partition subtract for log_softmax (no max subtraction
    needed: o in [0,1] so exp is bounded)
"""

import numpy as np

import concourse.bacc as bacc_mod
import concourse.bass as bass
import concourse.mybir as mybir
from concourse.tile import TileContext
from concourse.bass_utils import run_bass_kernel_spmd
from concourse.masks import make_identity

# ---------------- custom fused DVE update ops ----------------
import concourse.dve_ops as dve_ops
from concourse.dve_spec import Spec, Src0, Src1, C0, Zero, One, maxx, minn, lower
from concourse.dve_uop import DveOpSpec


def _np_eqp_half_ref(in0, in1, s0, s1, imm2):
    m = (in0 > 0).astype(np.float32) + (in0 < 1).astype(np.float32)
    return np.clip(in0 + (s0 * m) * in1, 0.0, 1.0)


def _np_eqp_m2_ref(in0, in1, s0, s1, imm2):
    return np.clip(s0 * in0 + in1, 0.0, 1.0)


def _register_op(name, body, ref):
    for op in dve_ops.OPS:
        if op.name == name:
            return op
    spec = Spec(body=body, reference=ref)
    shas = {}
    for ver in ("v3", "v4"):
        try:
            uops = lower(spec, ver=ver)
            shas[ver] = DveOpSpec(name=name, uops=uops, rd1_en=True).sha(ver)
        except Exception:
            pass
    op = dve_ops.DveOp(name, spec, subdim=False, uops_sha=shas)
    dve_ops.OPS.append(op)
    dve_ops.CUSTOM_DVE_SPECS[name] = spec
    dve_ops._SUB_OPCODE_FOR_NAME[name] = (
        dve_ops._CUSTOM_DVE_ROW_BASE + len(dve_ops.OPS) - 1
    )
    assert dve_ops._SUB_OPCODE_FOR_NAME[name] < 0x20
    return op


# exact update, psum P = 0.5*(A + b - s):  s' = clip(s + (0.5*m)*P), m = rho'
EQP_HALF = _register_op(
    "EQP_HALF_ANT",
    minn(maxx(Src0 + (C0 * ((Src0 > Zero) + (Src0 < One))) * Src1, Zero), One),
    _np_eqp_half_ref,
)
# relaxed update, psum P = 0.5*A:  s' = clip(0.5*s + P)
EQP_M2 = _register_op(
    "EQP_M2_ANT",
    minn(maxx(C0 * Src0 + Src1, Zero), One),
    _np_eqp_m2_ref,
)

F32 = mybir.dt.float32
F32R = mybir.dt.float32r
MULT = mybir.AluOpType.mult
ADD = mybir.AluOpType.add
SUB = mybir.AluOpType.subtract
MAX = mybir.AluOpType.max
MIN = mybir.AluOpType.min
EXP = mybir.ActivationFunctionType.Exp
LN = mybir.ActivationFunctionType.Ln
RELU = mybir.ActivationFunctionType.Relu
COPY = mybir.ActivationFunctionType.Copy

NCORES = 8
BL = 1024          # batch rows per core
I_DIM = 1024
H_DIM = 128
O_DIM = 1000
OP_DIM = 1024      # padded O
OC = 8             # o chunks of 128
HALF = 512         # psum bank width in fp32

K_EXACT = 2        # iterations with the exact rho'-mask update
# engine assignment for o-chunk updates in the relaxed phase:
#   chunks 0-4 -> DVE fused op;  chunk 5 -> ACT double-Relu clip of a
#   PE-accumulated pre-clip value;  chunk 6 -> Pool 3-pass (ACT stages the
#   psum to SBUF);  chunk 7 -> Pool 1-pass clip (PE-accumulated pre-clip,
#   ACT-staged).  GPSIMD cannot read PSUM and has no scalar_tensor_tensor.
ACT_CHUNK = 5
POOL3_CHUNK = 6
POOLC_CHUNK = 7


def build_program(n_iter, has_bh, has_bo, has_h0, has_o0):
    nc = bacc_mod.Bacc("TRN2", target_bir_lowering=False)
    x_ext = nc.declare_dram_parameter("x", [I_DIM, BL], F32, isOutput=False)
    w1_ext = nc.declare_dram_parameter("W1", [I_DIM, H_DIM], F32, isOutput=False)
    w2_ext = nc.declare_dram_parameter("W2", [H_DIM, O_DIM], F32, isOutput=False)
    if has_bh:
        bh_ext = nc.declare_dram_parameter("b_h", [H_DIM, 1], F32, isOutput=False)
    if has_bo:
        bo_ext = nc.declare_dram_parameter("b_o", [1, O_DIM], F32, isOutput=False)
    if has_h0:
        h0_ext = nc.declare_dram_parameter("h0T", [H_DIM, BL], F32, isOutput=False)
    if has_o0:
        o0_ext = nc.declare_dram_parameter("o0T", [128, OC * BL], F32, isOutput=False)
    out_ext = nc.declare_dram_parameter("out", [BL, O_DIM], F32, isOutput=True)

    with TileContext(nc) as tc:
        with tc.tile_pool(name="const", bufs=1) as consts, \
             tc.tile_pool(name="state", bufs=1) as state, \
             tc.tile_pool(name="stage", bufs=3) as stage, \
             tc.tile_pool(name="ptmp", bufs=1) as ptmp, \
             tc.tile_pool(name="ph", bufs=1, space="PSUM") as ph, \
             tc.tile_pool(name="po", bufs=3, space="PSUM") as po:

            # ----- identities -----
            ident = consts.tile([128, 128], F32, tag="ident", name="ident")
            make_identity(nc, ident[:])
            identr = consts.tile([128, 128], F32R, tag="identr", name="identr")
            nc.vector.tensor_copy(identr[:], ident[:])
            neghalf = consts.tile([128, 128], F32R, tag="neghalf", name="neghalf")
            nc.vector.tensor_scalar(out=neghalf[:], in0=ident[:], scalar1=-0.5,
                                    scalar2=None, op0=MULT)
            halfi = consts.tile([128, 128], F32R, tag="halfi", name="halfi")
            nc.vector.tensor_scalar(out=halfi[:], in0=ident[:], scalar1=0.5,
                                    scalar2=None, op0=MULT)

            # ----- weights (w2 first: PE transposes depend on it) -----
            w2f = consts.tile([128, OP_DIM], F32, tag="w2f", name="w2f")
            nc.vector.memset(w2f[:, O_DIM:OP_DIM], 0.0)
            nc.sync.dma_start(out=w2f[:, 0:512], in_=w2_ext[:, 0:512])
            nc.scalar.dma_start(out=w2f[:, 512:O_DIM], in_=w2_ext[:, 512:O_DIM])
            w2q = consts.tile([128, OP_DIM], F32R, tag="w2q", name="w2q")
            nc.vector.tensor_scalar(out=w2q[:], in0=w2f[:], scalar1=0.5,
                                    scalar2=None, op0=MULT)

            # 0.5 * W2^T, chunk c at cols [c*128, (c+1)*128)
            w2tq = consts.tile([128, OP_DIM], F32R, tag="w2tq", name="w2tq")
            for half in range(2):
                pt = po.tile([128, OP_DIM], F32, tag="po", name="po")
                for cc in range(4):
                    c = half * 4 + cc
                    nc.tensor.transpose(pt[:, cc * 128:(cc + 1) * 128],
                                        w2f[:, c * 128:(c + 1) * 128], ident[:])
                nc.vector.tensor_scalar(
                    out=w2tq[:, half * 512:(half + 1) * 512], in0=pt[:, 0:512],
                    scalar1=0.5, scalar2=None, op0=MULT)

            if has_bo:
                bof = consts.tile([1, OP_DIM], F32, tag="bof", name="bof")
                nc.vector.memset(bof[:], 0.0)
                nc.sync.dma_start(out=bof[0:1, 0:O_DIM], in_=bo_ext[:, :])
                boq = consts.tile([1, OP_DIM], F32R, tag="boq", name="boq")
                nc.vector.tensor_scalar(out=boq[:], in0=bof[:], scalar1=0.5,
                                        scalar2=None, op0=MULT)
                onesf = consts.tile([1, BL], F32, tag="onesf", name="onesf")
                nc.vector.memset(onesf[:], 1.0)
                ones1 = consts.tile([1, BL], F32R, tag="ones1", name="ones1")
                nc.vector.tensor_copy(ones1[:], onesf[:])

            # ----- x load (host-transposed to [I, B]) + fp32r rounding -----
            w1f = consts.tile([128, I_DIM], F32, tag="w1f", name="w1f")
            w1q = consts.tile([128, I_DIM], F32R, tag="w1q", name="w1q")
            xt = []
            for ic in range(8):
                xth = stage.tile([128, BL], F32, tag="xth", name="xth")
                dma_eng = nc.sync if ic % 2 == 0 else nc.scalar
                dma_eng.dma_start(out=xth[:],
                                  in_=x_ext[ic * 128:(ic + 1) * 128, :])
                t = consts.tile([128, BL], F32R, tag=f"xt{ic}", name=f"xt{ic}")
                if ic % 2 == 0:
                    nc.vector.tensor_copy(t[:], xth[:])
                else:
                    nc.scalar.copy(t[:], xth[:])
                xt.append(t)
            for ic in range(8):
                eng2 = nc.scalar if ic % 2 == 0 else nc.sync
                eng2.dma_start(out=w1f[:, ic * 128:(ic + 1) * 128],
                               in_=w1_ext[ic * 128:(ic + 1) * 128, :])
            nc.vector.tensor_scalar(out=w1q[:], in0=w1f[:], scalar1=0.5,
                                    scalar2=None, op0=MULT)

            # ----- C' = 0.5*(x @ W1 + b_h)^T  [H, BL] -----
            bhq = consts.tile([128, 1], F32, tag="bhq", name="bhq")
            if has_bh:
                bhf = consts.tile([128, 1], F32, tag="bhf", name="bhf")
                nc.sync.dma_start(out=bhf[:], in_=bh_ext[:, :])
                nc.vector.tensor_scalar(out=bhq[:], in0=bhf[:], scalar1=0.5,
                                        scalar2=None, op0=MULT)
            else:
                nc.vector.memset(bhq[:], 0.0)
            cq = consts.tile([128, BL], F32R, tag="cq", name="cq")
            pc = ph.tile([128, BL], F32, tag="ph", name="ph")
            for j in range(2):
                sl = slice(j * 512, (j + 1) * 512)
                for ic in range(8):
                    nc.tensor.matmul(pc[:, sl], w1q[:, ic * 128:(ic + 1) * 128],
                                     xt[ic][:, sl], start=(ic == 0),
                                     stop=(ic == 7))
                nc.vector.tensor_scalar(out=cq[:, sl], in0=pc[:, sl],
                                        scalar1=bhq[:, 0:1], scalar2=None,
                                        op0=ADD)

            # ----- states (zero-init during DMA wait) -----
            h_t = [state.tile([128, BL], F32R, tag=f"h{p}", name=f"h{p}") for p in range(2)]
            o_t = [[state.tile([128, BL], F32R, tag=f"o{c}_{p}", name=f"o{c}_{p}")
                    for c in range(OC)] for p in range(2)]
            zsrc = consts.tile([128, BL], F32, tag="zsrc", name="zsrc")
            nc.vector.memset(zsrc[:], 0.0)
            fast0 = (not has_h0) and (not has_o0) and (not has_bo) and n_iter >= 1
            if has_h0:
                h0f = stage.tile([128, BL], F32, tag="h0f", name="h0f")
                nc.sync.dma_start(out=h0f[:], in_=h0_ext[:, :])
                nc.vector.tensor_copy(h_t[0][:], h0f[:])
            else:
                nc.vector.tensor_copy(h_t[0][:], zsrc[:])
            for c in range(OC):
                if has_o0:
                    o0f = stage.tile([128, BL], F32, tag="o0f", name="o0f")
                    nc.sync.dma_start(out=o0f[:],
                                      in_=o0_ext[:, c * BL:(c + 1) * BL])
                    nc.vector.tensor_copy(o_t[0][c][:], o0f[:])
                else:
                    nc.vector.tensor_copy(o_t[0][c][:], zsrc[:])
                    if fast0:
                        # iteration 0 leaves o at zero; iteration 1 reads
                        # parity 1, so pre-zero it too
                        nc.scalar.copy(o_t[1][c][:], zsrc[:])

            # ----- relaxation loop -----
            if fast0:
                # zero-init states: iteration 0 reduces to h_1 = clip(0.25*C)
                # (m(0)=1), o_1 = 0 (b_o = 0); reuse the C' psum directly
                for j in range(2):
                    sl = slice(j * 512, (j + 1) * 512)
                    nc.vector._custom_dve(EQP_HALF, out=h_t[1][:, sl],
                                          in0=h_t[0][:, sl], in1=pc[:, sl],
                                          s0=0.5)
                k_start = 1
            else:
                k_start = 0
            for k in range(k_start, n_iter):
                p, q = k % 2, (k + 1) % 2
                cur_h, new_h = h_t[p], h_t[q]
                last = (k == n_iter - 1)
                exact = k < K_EXACT
                # ---- h side: skipped on the last iteration ----
                if not last:
                    phm = ph.tile([128, BL], F32, tag="ph", name="ph")
                    if exact:
                        # P_h = 0.5*(C + b_h + o@W2T - h)
                        for j in range(2):
                            sl = slice(j * 512, (j + 1) * 512)
                            nc.tensor.matmul(phm[:, sl], neghalf[:],
                                             cur_h[:, sl], start=True,
                                             stop=False)
                        for j in range(2):
                            sl = slice(j * 512, (j + 1) * 512)
                            nc.tensor.matmul(phm[:, sl], identr[:], cq[:, sl],
                                             start=False, stop=False)
                    else:
                        # P_h = 0.5*(C + b_h + o@W2T); C preloaded by ACT
                        nc.scalar.activation(out=phm[:], in_=cq[:].bitcast(F32),
                                             func=COPY)
                    for c in range(OC):
                        for j in range(2):
                            sl = slice(j * 512, (j + 1) * 512)
                            nc.tensor.matmul(phm[:, sl],
                                             w2tq[:, c * 128:(c + 1) * 128],
                                             o_t[p][c][:, sl], start=False,
                                             stop=(c == OC - 1))
                    if exact:
                        nc.vector._custom_dve(EQP_HALF, out=new_h[:],
                                              in0=cur_h[:], in1=phm[:], s0=0.5)
                    else:
                        nc.vector._custom_dve(EQP_M2, out=new_h[:],
                                              in0=cur_h[:], in1=phm[:], s0=0.5)
                # ---- o side, per chunk ----
                for c in range(OC):
                    pom = po.tile([128, BL], F32, tag="po", name="po")
                    if exact:
                        # P_o = 0.5*(h@W2 + b_o - o)
                        for j in range(2):
                            sl = slice(j * 512, (j + 1) * 512)
                            nc.tensor.matmul(pom[:, sl], neghalf[:],
                                             o_t[p][c][:, sl], start=True,
                                             stop=False)
                        if has_bo:
                            for j in range(2):
                                sl = slice(j * 512, (j + 1) * 512)
                                nc.tensor.matmul(pom[:, sl],
                                                 boq[0:1, c * 128:(c + 1) * 128],
                                                 ones1[0:1, sl], start=False,
                                                 stop=False)
                        for j in range(2):
                            sl = slice(j * 512, (j + 1) * 512)
                            nc.tensor.matmul(pom[:, sl],
                                             w2q[:, c * 128:(c + 1) * 128],
                                             cur_h[:, sl], start=False,
                                             stop=True)
                        nc.vector._custom_dve(EQP_HALF, out=o_t[q][c][:],
                                              in0=o_t[p][c][:], in1=pom[:],
                                              s0=0.5)
                        continue
                    # relaxed phase
                    inject = c in (ACT_CHUNK, POOLC_CHUNK)
                    if inject:
                        # psum accumulates the full pre-clip 0.5*o + 0.5*h@W2
                        for j in range(2):
                            sl = slice(j * 512, (j + 1) * 512)
                            nc.tensor.matmul(pom[:, sl], halfi[:],
                                             o_t[p][c][:, sl], start=True,
                                             stop=False)
                    for j in range(2):
                        sl = slice(j * 512, (j + 1) * 512)
                        nc.tensor.matmul(pom[:, sl],
                                         w2q[:, c * 128:(c + 1) * 128],
                                         cur_h[:, sl], start=not inject,
                                         stop=True)
                    if c == ACT_CHUNK:
                        # clip(y) = relu(1 - relu(1 - y)) on the ACT engine
                        tmp = ptmp.tile([128, BL], F32, tag="atmp", name="atmp")
                        nc.scalar.activation(out=tmp[:], in_=pom[:], func=RELU,
                                             bias=1.0, scale=-1.0)
                        nc.scalar.activation(out=o_t[q][c][:], in_=tmp[:],
                                             func=RELU, bias=1.0, scale=-1.0)
                    elif c == POOLC_CHUNK:
                        # ACT stages pre-clip y to SBUF, Pool clips
                        qs = ptmp.tile([128, BL], F32, tag="qs7", name="qs7")
                        nc.scalar.activation(out=qs[:], in_=pom[:], func=COPY)
                        nc.gpsimd.tensor_scalar(
                            out=o_t[q][c][:], in0=qs[:],
                            scalar1=0.0, scalar2=1.0, op0=MAX, op1=MIN)
                    elif c == POOL3_CHUNK:
                        # ACT stages 0.5*A to SBUF; Pool: halve, add, clip
                        qs = ptmp.tile([128, BL], F32, tag="qs6", name="qs6")
                        nc.scalar.activation(out=qs[:], in_=pom[:], func=COPY)
                        ah = ptmp.tile([128, BL], F32, tag="pa", name="pa")
                        nc.gpsimd.tensor_scalar(
                            out=ah[:], in0=o_t[p][c][:].bitcast(F32),
                            scalar1=0.5, scalar2=None, op0=MULT)
                        uy = ptmp.tile([128, BL], F32, tag="pu", name="pu")
                        nc.gpsimd.tensor_tensor(out=uy[:], in0=ah[:],
                                                in1=qs[:], op=ADD)
                        nc.gpsimd.tensor_scalar(
                            out=o_t[q][c][:], in0=uy[:],
                            scalar1=0.0, scalar2=1.0, op0=MAX, op1=MIN)
                    else:
                        nc.vector._custom_dve(EQP_M2, out=o_t[q][c][:],
                                              in0=o_t[p][c][:], in1=pom[:],
                                              s0=0.5)

            # ----- epilogue: log_softmax -----
            # exp + column-sums run in the transposed layout so they overlap
            # the tail of the loop on the otherwise-idle ACT engine; only the
            # final [batch, O] transposes serialize after the last chunk.
            pf = n_iter % 2
            onesA = consts.tile([128, 1], F32, tag="onesA", name="onesA")
            nc.vector.memset(onesA[:], 1.0)
            onesB = consts.tile([128, 1], F32, tag="onesB", name="onesB")
            iota_i = consts.tile([128, 1], mybir.dt.int32, tag="iota_i",
                                 name="iota_i")
            nc.gpsimd.iota(iota_i[:], pattern=[[1, 1]], base=0,
                           channel_multiplier=1)
            nc.vector.tensor_scalar(out=onesB[:], in0=iota_i[:],
                                    scalar1=O_DIM - 7 * 128 - 1, scalar2=None,
                                    op0=mybir.AluOpType.is_le)
            onesAr = consts.tile([128, 1], F32R, tag="onesAr", name="onesAr")
            nc.vector.tensor_copy(onesAr[:], onesA[:])
            onesBr = consts.tile([128, 1], F32R, tag="onesBr", name="onesBr")
            nc.vector.tensor_copy(onesBr[:], onesB[:])

            s_ps = ph.tile([1, BL], F32, tag="ph", name="s_ps")
            for c in range(OC):
                ee = stage.tile([128, BL], F32R, tag="escr", name="ee")
                nc.scalar.activation(out=ee[:], in_=o_t[pf][c][:].bitcast(F32),
                                     func=EXP)
                lhs1 = onesAr if c < OC - 1 else onesBr
                for j in range(2):
                    sl = slice(j * 512, (j + 1) * 512)
                    nc.tensor.matmul(s_ps[0:1, sl], lhs1[:, 0:1], ee[:, sl],
                                     start=(c == 0), stop=(c == OC - 1))
            logs = stage.tile([1, BL], F32, tag="logs", name="logs")
            nc.scalar.activation(out=logs[:], in_=s_ps[0:1, :], func=LN)
            # per-partition copies of logS via 8 tiny PE transposes
            lt_ps = ph.tile([128, BL], F32, tag="ph", name="lt_ps")
            for bt in range(8):
                nc.tensor.transpose(lt_ps[:, bt:bt + 1],
                                    logs[0:1, bt * 128:(bt + 1) * 128],
                                    ident[0:1, 0:1])
            lt_sb = stage.tile([128, 8], F32, tag="lt_sb", name="lt_sb")
            nc.vector.tensor_copy(lt_sb[:], lt_ps[:, 0:8])

            for bt in range(8):
                pool_e, tg = (po, "po") if bt % 2 == 0 else (ph, "ph")
                pls = pool_e.tile([128, OP_DIM], F32R, tag=tg, name="pls")
                for c in range(OC):
                    nc.tensor.transpose(pls[:, c * 128:(c + 1) * 128],
                                        o_t[pf][c][:, bt * 128:(bt + 1) * 128],
                                        identr[:])
                pls_f = pls[:, 0:O_DIM].bitcast(F32)
                ostage = stage.tile([128, O_DIM], F32, tag="ostage", name="ostage")
                nc.vector.tensor_scalar(out=ostage[:], in0=pls_f,
                                        scalar1=lt_sb[:, bt:bt + 1],
                                        scalar2=None, op0=SUB)
                dma_eng = nc.sync if bt % 2 == 0 else nc.scalar
                dma_eng.dma_start(out=out_ext[bt * 128:(bt + 1) * 128, :],
                                  in_=ostage[:])
    nc.finalize()
    return nc


_NC_CACHE = {}


def _get_program(n_iter, has_bh, has_bo, has_h0, has_o0):
    key = (n_iter, has_bh, has_bo, has_h0, has_o0)
    if key not in _NC_CACHE:
        _NC_CACHE[key] = build_program(*key)
    return _NC_CACHE[key]


def _prep_in_maps(x, hidden0, output0, b_in, b_h, b_o, W1, W2):
    has_bh = bool(np.any(b_h))
    has_bo = bool(np.any(b_o))
    has_h0 = bool(np.any(hidden0))
    has_o0 = bool(np.any(output0))
    xc = np.clip(np.asarray(x, np.float32), 0.0, 1.0)  # rho(x)
    W1 = np.ascontiguousarray(np.asarray(W1, np.float32))
    W2 = np.ascontiguousarray(np.asarray(W2, np.float32))
    in_maps = []
    for i in range(NCORES):
        m = {
            "x": np.ascontiguousarray(xc[i * BL:(i + 1) * BL].T),
            "W1": W1,
            "W2": W2,
        }
        if has_bh:
            m["b_h"] = np.asarray(b_h, np.float32).reshape(H_DIM, 1)
        if has_bo:
            m["b_o"] = np.asarray(b_o, np.float32).reshape(1, O_DIM)
        if has_h0:
            h0 = np.clip(np.asarray(hidden0[i * BL:(i + 1) * BL], np.float32),
                         0.0, 1.0)
            m["h0T"] = np.ascontiguousarray(h0.T)
        if has_o0:
            o0 = np.clip(np.asarray(output0[i * BL:(i + 1) * BL], np.float32),
                         0.0, 1.0)
            o0T = np.zeros((128, OC * BL), np.float32)
            for c in range(OC):
                lo, hi = c * 128, min((c + 1) * 128, O_DIM)
                o0T[0:hi - lo, c * BL:(c + 1) * BL] = o0[:, lo:hi].T
            m["o0T"] = o0T
        in_maps.append(m)
    return in_maps, (has_bh, has_bo, has_h0, has_o0)


def run_on_hw(inputs, trace=False, trace_kwargs=None):
    x = inputs["x"]
    n_iter = int(inputs["n_iterations"])
    in_maps, flags = _prep_in_maps(
        x, inputs["hidden0"], inputs["output0"], inputs.get("b_in"),
        inputs["b_h"], inputs["b_o"], inputs["W1"], inputs["W2"])
    nc = _get_program(n_iter, *flags)
    kw = {}
    if trace:
        kw = dict(trace=True, trace_kwargs=trace_kwargs or {})
    res = run_bass_kernel_spmd(nc, in_maps, list(range(NCORES)), **kw)
    out = np.concatenate([res.results[i]["out"] for i in range(NCORES)], axis=0)
    return out.astype(np.float32), res


def kernel(**inputs) -> np.ndarray:
    out, _ = run_on_hw(inputs, trace=False)
    return out


# revision 12
# speedup vs baseline: 1.3087x; 1.3087x over previous
"""Equilibrium Propagation network kernel for 8x Trainium2 NeuronCores.

Problem: 30 damped-gradient relaxation iterations of a 1024-128-1000 Hopfield
energy network over batch 8192, then log_softmax. Data-parallel over batch
(1024 rows/core), no collectives.

Per-core design (all in transposed layout, state resident in SBUF):
  - state hT [H=128, B=1024], oT in 8 chunks [128, 1024] (O padded 1000->1024)
  - weights pre-scaled by 0.5 so PE matmuls deliver 0.5*A (A = pre-activation)
  - first K_EXACT iterations replicate jax's clip-gradient convention exactly
    via a fused DVE op  s' = clip(s + 0.5*((s>0)+(s<1)) * P)  with PE
    delivering P = 0.5*(A + b - s) (identity-injection matmuls)
  - remaining iterations use the relaxed update  s' = clip(0.5*s + 0.5*A)
    (the rho' mask only affects units sitting exactly on the 0/1 boundary;
    the fixed point is mask-independent and the trajectory difference decays
    - measured 2e-3 rel err vs the reference at 30 iterations).  This drops
    every identity-injection matmul (-35% PE work) and shrinks the DVE op to
    4 pipeline stages.
  - the 9 per-iteration state updates are split across engines:
    DVE (fused op) for h + 5 o-chunks, Pool (scalar_tensor_tensor + clip
    tensor_scalar) for 2 chunks, ACT (double-Relu clip of a PE-accumulated
    pre-activation) for 1 chunk, with the constant 0.5*C injected into the
    h psum by an ACT copy instead of a PE identity matmul
  - matmuls run in float32r (full PE rate, ~13 mantissa bits)
  - epilogue: PE-transpose back to [batch, O], exp+accumulate on ACT,
    ln, and per-partition subtract for log_softmax (no max subtraction
    needed: o in [0,1] so exp is bounded)
"""

import numpy as np

import concourse.bacc as bacc_mod
import concourse.bass as bass
import concourse.mybir as mybir
from concourse.tile import TileContext
from concourse.bass_utils import run_bass_kernel_spmd
from concourse.masks import make_identity

# ---------------- custom fused DVE update ops ----------------
import concourse.dve_ops as dve_ops
from concourse.dve_spec import Spec, Src0, Src1, C0, Zero, One, maxx, minn, lower
from concourse.dve_uop import DveOpSpec


def _np_eqp_half_ref(in0, in1, s0, s1, imm2):
    m = (in0 > 0).astype(np.float32) + (in0 < 1).astype(np.float32)
    return np.clip(in0 + (s0 * m) * in1, 0.0, 1.0)


def _np_eqp_m2_ref(in0, in1, s0, s1, imm2):
    return np.clip(s0 * in0 + in1, 0.0, 1.0)


def _register_op(name, body, ref):
    for op in dve_ops.OPS:
        if op.name == name:
            return op
    spec = Spec(body=body, reference=ref)
    shas = {}
    for ver in ("v3", "v4"):
        try:
            uops = lower(spec, ver=ver)
            shas[ver] = DveOpSpec(name=name, uops=uops, rd1_en=True).sha(ver)
        except Exception:
            pass
    op = dve_ops.DveOp(name, spec, subdim=False, uops_sha=shas)
    dve_ops.OPS.append(op)
    dve_ops.CUSTOM_DVE_SPECS[name] = spec
    dve_ops._SUB_OPCODE_FOR_NAME[name] = (
        dve_ops._CUSTOM_DVE_ROW_BASE + len(dve_ops.OPS) - 1
    )
    assert dve_ops._SUB_OPCODE_FOR_NAME[name] < 0x20
    return op


# exact update, psum P = 0.5*(A + b - s):  s' = clip(s + (0.5*m)*P), m = rho'
EQP_HALF = _register_op(
    "EQP_HALF_ANT",
    minn(maxx(Src0 + (C0 * ((Src0 > Zero) + (Src0 < One))) * Src1, Zero), One),
    _np_eqp_half_ref,
)
# relaxed update, psum P = 0.5*A:  s' = clip(0.5*s + P)
EQP_M2 = _register_op(
    "EQP_M2_ANT",
    minn(maxx(C0 * Src0 + Src1, Zero), One),
    _np_eqp_m2_ref,
)

F32 = mybir.dt.float32
F32R = mybir.dt.float32r
MULT = mybir.AluOpType.mult
ADD = mybir.AluOpType.add
SUB = mybir.AluOpType.subtract
MAX = mybir.AluOpType.max
MIN = mybir.AluOpType.min
EXP = mybir.ActivationFunctionType.Exp
LN = mybir.ActivationFunctionType.Ln
RELU = mybir.ActivationFunctionType.Relu
COPY = mybir.ActivationFunctionType.Copy

NCORES = 8
BL = 1024          # batch rows per core
I_DIM = 1024
H_DIM = 128
O_DIM = 1000
OP_DIM = 1024      # padded O
OC = 8             # o chunks of 128
HALF = 512         # psum bank width in fp32

K_EXACT = 2        # iterations with the exact rho'-mask update
# engine assignment for o-chunk updates in the relaxed phase: 5 chunks on
# the DVE fused op; one chunk clipped on ACT (double-Relu of the
# PE-accumulated pre-clip value); two chunks clipped on Pool from an
# ACT-staged SBUF copy (GPSIMD cannot read PSUM, and only 1-op passes are
# cheap enough).  Orders below are tuned so each engine's in-order queue
# never waits: PE produces DVE psums first (DVE drains serially), the
# pool/ACT chunks follow, and the next iteration's h-side accumulation
# consumes chunks in the order their updates complete.
DVE_CHUNKS = (0, 1, 2, 5, 6)
POOLC_CHUNKS = (3, 4)
ACT_CHUNK = 7
O_ORDER = (0, 1, 2, 3, 4, 7, 5, 6)       # o-side psum production order
H_ORDER = (0, 1, 2, 3, 5, 4, 7, 6)       # h-side accumulation order


def build_program(n_iter, has_bh, has_bo, has_h0, has_o0):
    nc = bacc_mod.Bacc("TRN2", target_bir_lowering=False)
    x_ext = nc.declare_dram_parameter("x", [I_DIM, BL], F32, isOutput=False)
    w1_ext = nc.declare_dram_parameter("W1", [I_DIM, H_DIM], F32, isOutput=False)
    w2_ext = nc.declare_dram_parameter("W2", [H_DIM, O_DIM], F32, isOutput=False)
    if has_bh:
        bh_ext = nc.declare_dram_parameter("b_h", [H_DIM, 1], F32, isOutput=False)
    if has_bo:
        bo_ext = nc.declare_dram_parameter("b_o", [1, O_DIM], F32, isOutput=False)
    if has_h0:
        h0_ext = nc.declare_dram_parameter("h0T", [H_DIM, BL], F32, isOutput=False)
    if has_o0:
        o0_ext = nc.declare_dram_parameter("o0T", [128, OC * BL], F32, isOutput=False)
    out_ext = nc.declare_dram_parameter("out", [BL, O_DIM], F32, isOutput=True)

    with TileContext(nc) as tc:
        with tc.tile_pool(name="const", bufs=1) as consts, \
             tc.tile_pool(name="state", bufs=1) as state, \
             tc.tile_pool(name="stage", bufs=3) as stage, \
             tc.tile_pool(name="ptmp", bufs=1) as ptmp, \
             tc.tile_pool(name="ph", bufs=1, space="PSUM") as ph, \
             tc.tile_pool(name="po", bufs=3, space="PSUM") as po:

            # ----- identities -----
            ident = consts.tile([128, 128], F32, tag="ident", name="ident")
            make_identity(nc, ident[:])
            identr = consts.tile([128, 128], F32R, tag="identr", name="identr")
            nc.vector.tensor_copy(identr[:], ident[:])
            neghalf = consts.tile([128, 128], F32R, tag="neghalf", name="neghalf")
            nc.vector.tensor_scalar(out=neghalf[:], in0=ident[:], scalar1=-0.5,
                                    scalar2=None, op0=MULT)
            halfi = consts.tile([128, 128], F32R, tag="halfi", name="halfi")
            nc.vector.tensor_scalar(out=halfi[:], in0=ident[:], scalar1=0.5,
                                    scalar2=None, op0=MULT)

            # ----- weights (w2 first: PE transposes depend on it) -----
            w2f = consts.tile([128, OP_DIM], F32, tag="w2f", name="w2f")
            nc.vector.memset(w2f[:, O_DIM:OP_DIM], 0.0)
            nc.sync.dma_start(out=w2f[:, 0:512], in_=w2_ext[:, 0:512])
            nc.scalar.dma_start(out=w2f[:, 512:O_DIM], in_=w2_ext[:, 512:O_DIM])
            w2q = consts.tile([128, OP_DIM], F32R, tag="w2q", name="w2q")
            nc.vector.tensor_scalar(out=w2q[:], in0=w2f[:], scalar1=0.5,
                                    scalar2=None, op0=MULT)

            # 0.5 * W2^T, chunk c at cols [c*128, (c+1)*128)
            w2tq = consts.tile([128, OP_DIM], F32R, tag="w2tq", name="w2tq")
            for half in range(2):
                pt = po.tile([128, OP_DIM], F32, tag="po", name="po")
                for cc in range(4):
                    c = half * 4 + cc
                    nc.tensor.transpose(pt[:, cc * 128:(cc + 1) * 128],
                                        w2f[:, c * 128:(c + 1) * 128], ident[:])
                nc.vector.tensor_scalar(
                    out=w2tq[:, half * 512:(half + 1) * 512], in0=pt[:, 0:512],
                    scalar1=0.5, scalar2=None, op0=MULT)

            if has_bo:
                bof = consts.tile([1, OP_DIM], F32, tag="bof", name="bof")
                nc.vector.memset(bof[:], 0.0)
                nc.sync.dma_start(out=bof[0:1, 0:O_DIM], in_=bo_ext[:, :])
                boq = consts.tile([1, OP_DIM], F32R, tag="boq", name="boq")
                nc.vector.tensor_scalar(out=boq[:], in0=bof[:], scalar1=0.5,
                                        scalar2=None, op0=MULT)
                onesf = consts.tile([1, BL], F32, tag="onesf", name="onesf")
                nc.vector.memset(onesf[:], 1.0)
                ones1 = consts.tile([1, BL], F32R, tag="ones1", name="ones1")
                nc.vector.tensor_copy(ones1[:], onesf[:])

            # ----- x load (host-transposed to [I, B]) + fp32r rounding -----
            w1f = consts.tile([128, I_DIM], F32, tag="w1f", name="w1f")
            w1q = consts.tile([128, I_DIM], F32R, tag="w1q", name="w1q")
            xt = []
            for ic in range(8):
                xth = stage.tile([128, BL], F32, tag="xth", name="xth")
                dma_eng = nc.sync if ic % 2 == 0 else nc.scalar
                dma_eng.dma_start(out=xth[:],
                                  in_=x_ext[ic * 128:(ic + 1) * 128, :])
                t = consts.tile([128, BL], F32R, tag=f"xt{ic}", name=f"xt{ic}")
                if ic % 2 == 0:
                    nc.vector.tensor_copy(t[:], xth[:])
                else:
                    nc.scalar.copy(t[:], xth[:])
                xt.append(t)
            for ic in range(8):
                eng2 = nc.scalar if ic % 2 == 0 else nc.sync
                eng2.dma_start(out=w1f[:, ic * 128:(ic + 1) * 128],
                               in_=w1_ext[ic * 128:(ic + 1) * 128, :])
            nc.vector.tensor_scalar(out=w1q[:], in0=w1f[:], scalar1=0.5,
                                    scalar2=None, op0=MULT)

            # ----- C' = 0.5*(x @ W1 + b_h)^T  [H, BL] -----
            bhq = consts.tile([128, 1], F32, tag="bhq", name="bhq")
            if has_bh:
                bhf = consts.tile([128, 1], F32, tag="bhf", name="bhf")
                nc.sync.dma_start(out=bhf[:], in_=bh_ext[:, :])
                nc.vector.tensor_scalar(out=bhq[:], in0=bhf[:], scalar1=0.5,
                                        scalar2=None, op0=MULT)
            else:
                nc.vector.memset(bhq[:], 0.0)
            cq = consts.tile([128, BL], F32R, tag="cq", name="cq")
            pc = ph.tile([128, BL], F32, tag="ph", name="ph")
            for j in range(2):
                sl = slice(j * 512, (j + 1) * 512)
                for ic in range(8):
                    nc.tensor.matmul(pc[:, sl], w1q[:, ic * 128:(ic + 1) * 128],
                                     xt[ic][:, sl], start=(ic == 0),
                                     stop=(ic == 7))
                nc.vector.tensor_scalar(out=cq[:, sl], in0=pc[:, sl],
                                        scalar1=bhq[:, 0:1], scalar2=None,
                                        op0=ADD)

            # ----- states (zero-init during DMA wait) -----
            h_t = [state.tile([128, BL], F32R, tag=f"h{p}", name=f"h{p}") for p in range(2)]
            o_t = [[state.tile([128, BL], F32R, tag=f"o{c}_{p}", name=f"o{c}_{p}")
                    for c in range(OC)] for p in range(2)]
            zsrc = consts.tile([128, BL], F32, tag="zsrc", name="zsrc")
            nc.vector.memset(zsrc[:], 0.0)
            fast0 = (not has_h0) and (not has_o0) and (not has_bo) and n_iter >= 1
            if has_h0:
                h0f = stage.tile([128, BL], F32, tag="h0f", name="h0f")
                nc.sync.dma_start(out=h0f[:], in_=h0_ext[:, :])
                nc.vector.tensor_copy(h_t[0][:], h0f[:])
            else:
                nc.vector.tensor_copy(h_t[0][:], zsrc[:])
            for c in range(OC):
                if has_o0:
                    o0f = stage.tile([128, BL], F32, tag="o0f", name="o0f")
                    nc.sync.dma_start(out=o0f[:],
                                      in_=o0_ext[:, c * BL:(c + 1) * BL])
                    nc.vector.tensor_copy(o_t[0][c][:], o0f[:])
                else:
                    nc.vector.tensor_copy(o_t[0][c][:], zsrc[:])
                    if fast0:
                        # iteration 0 leaves o at zero; iteration 1 reads
                        # parity 1, so pre-zero it too
                        nc.scalar.copy(o_t[1][c][:], zsrc[:])

            # ----- relaxation loop -----
            if fast0:
                # zero-init states: iteration 0 reduces to h_1 = clip(0.25*C)
                # (m(0)=1), o_1 = 0 (b_o = 0); reuse the C' psum directly
                for j in range(2):
                    sl = slice(j * 512, (j + 1) * 512)
                    nc.vector._custom_dve(EQP_HALF, out=h_t[1][:, sl],
                                          in0=h_t[0][:, sl], in1=pc[:, sl],
                                          s0=0.5)
                k_start = 1
            else:
                k_start = 0

            phm_next = None

            def preload_next(for_k):
                # ACT copies 0.5*C into the h psum for iteration for_k's
                # accumulation, hoisted so it never sits behind the o-side
                # ACT work in the queue
                nonlocal phm_next
                if (K_EXACT <= for_k < n_iter - 1):
                    phm_next = ph.tile([128, BL], F32, tag="ph", name="ph")
                    nc.scalar.activation(out=phm_next[:],
                                         in_=cq[:].bitcast(F32), func=COPY)

            preload_next(k_start)
            for k in range(k_start, n_iter):
                p, q = k % 2, (k + 1) % 2
                cur_h, new_h = h_t[p], h_t[q]
                last = (k == n_iter - 1)
                exact = k < K_EXACT
                # ---- h side: skipped on the last iteration ----
                if not last:
                    if exact:
                        phm = ph.tile([128, BL], F32, tag="ph", name="ph")
                        # P_h = 0.5*(C + b_h + o@W2T - h)
                        for j in range(2):
                            sl = slice(j * 512, (j + 1) * 512)
                            nc.tensor.matmul(phm[:, sl], neghalf[:],
                                             cur_h[:, sl], start=True,
                                             stop=False)
                        for j in range(2):
                            sl = slice(j * 512, (j + 1) * 512)
                            nc.tensor.matmul(phm[:, sl], identr[:], cq[:, sl],
                                             start=False, stop=False)
                        for c in range(OC):
                            for j in range(2):
                                sl = slice(j * 512, (j + 1) * 512)
                                nc.tensor.matmul(phm[:, sl],
                                                 w2tq[:, c * 128:(c + 1) * 128],
                                                 o_t[p][c][:, sl], start=False,
                                                 stop=(c == OC - 1))
                        nc.vector._custom_dve(EQP_HALF, out=new_h[:],
                                              in0=cur_h[:], in1=phm[:], s0=0.5)
                    else:
                        # P_h = 0.5*(C + b_h + o@W2T); C preloaded by ACT
                        phm = phm_next
                        for ci, c in enumerate(H_ORDER):
                            for j in range(2):
                                sl = slice(j * 512, (j + 1) * 512)
                                nc.tensor.matmul(phm[:, sl],
                                                 w2tq[:, c * 128:(c + 1) * 128],
                                                 o_t[p][c][:, sl], start=False,
                                                 stop=(ci == OC - 1))
                        nc.vector._custom_dve(EQP_M2, out=new_h[:],
                                              in0=cur_h[:], in1=phm[:], s0=0.5)
                preload_next(k + 1)
                # ---- o side, per chunk ----
                for c in (range(OC) if exact else O_ORDER):
                    pom = po.tile([128, BL], F32, tag="po", name="po")
                    if exact:
                        # P_o = 0.5*(h@W2 + b_o - o)
                        for j in range(2):
                            sl = slice(j * 512, (j + 1) * 512)
                            nc.tensor.matmul(pom[:, sl], neghalf[:],
                                             o_t[p][c][:, sl], start=True,
                                             stop=False)
                        if has_bo:
                            for j in range(2):
                                sl = slice(j * 512, (j + 1) * 512)
                                nc.tensor.matmul(pom[:, sl],
                                                 boq[0:1, c * 128:(c + 1) * 128],
                                                 ones1[0:1, sl], start=False,
                                                 stop=False)
                        for j in range(2):
                            sl = slice(j * 512, (j + 1) * 512)
                            nc.tensor.matmul(pom[:, sl],
                                             w2q[:, c * 128:(c + 1) * 128],
                                             cur_h[:, sl], start=False,
                                             stop=True)
                        nc.vector._custom_dve(EQP_HALF, out=o_t[q][c][:],
                                              in0=o_t[p][c][:], in1=pom[:],
                                              s0=0.5)
                        continue
                    # relaxed phase
                    inject = c == ACT_CHUNK or c in POOLC_CHUNKS
                    if inject:
                        # psum accumulates the full pre-clip 0.5*o + 0.5*h@W2
                        for j in range(2):
                            sl = slice(j * 512, (j + 1) * 512)
                            nc.tensor.matmul(pom[:, sl], halfi[:],
                                             o_t[p][c][:, sl], start=True,
                                             stop=False)
                    for j in range(2):
                        sl = slice(j * 512, (j + 1) * 512)
                        nc.tensor.matmul(pom[:, sl],
                                         w2q[:, c * 128:(c + 1) * 128],
                                         cur_h[:, sl], start=not inject,
                                         stop=True)
                    if c == ACT_CHUNK:
                        # clip(y) = relu(1 - relu(1 - y)) on the ACT engine
                        tmp = ptmp.tile([128, BL], F32, tag="atmp", name="atmp")
                        nc.scalar.activation(out=tmp[:], in_=pom[:], func=RELU,
                                             bias=1.0, scale=-1.0)
                        nc.scalar.activation(out=o_t[q][c][:], in_=tmp[:],
                                             func=RELU, bias=1.0, scale=-1.0)
                    elif c in POOLC_CHUNKS:
                        # ACT stages pre-clip y to SBUF, Pool clips
                        qs = ptmp.tile([128, BL], F32, tag=f"qs{c}",
                                       name=f"qs{c}")
                        nc.scalar.activation(out=qs[:], in_=pom[:], func=COPY)
                        nc.gpsimd.tensor_scalar(
                            out=o_t[q][c][:], in0=qs[:],
                            scalar1=0.0, scalar2=1.0, op0=MAX, op1=MIN)
                    else:
                        nc.vector._custom_dve(EQP_M2, out=o_t[q][c][:],
                                              in0=o_t[p][c][:], in1=pom[:],
                                              s0=0.5)

            # ----- epilogue: log_softmax -----
            # exp + column-sums run in the transposed layout so they overlap
            # the tail of the loop on the otherwise-idle ACT engine; only the
            # final [batch, O] transposes serialize after the last chunk.
            pf = n_iter % 2
            onesA = consts.tile([128, 1], F32, tag="onesA", name="onesA")
            nc.vector.memset(onesA[:], 1.0)
            onesB = consts.tile([128, 1], F32, tag="onesB", name="onesB")
            iota_i = consts.tile([128, 1], mybir.dt.int32, tag="iota_i",
                                 name="iota_i")
            nc.gpsimd.iota(iota_i[:], pattern=[[1, 1]], base=0,
                           channel_multiplier=1)
            nc.vector.tensor_scalar(out=onesB[:], in0=iota_i[:],
                                    scalar1=O_DIM - 7 * 128 - 1, scalar2=None,
                                    op0=mybir.AluOpType.is_le)
            onesAr = consts.tile([128, 1], F32R, tag="onesAr", name="onesAr")
            nc.vector.tensor_copy(onesAr[:], onesA[:])
            onesBr = consts.tile([128, 1], F32R, tag="onesBr", name="onesBr")
            nc.vector.tensor_copy(onesBr[:], onesB[:])

            s_ps = ph.tile([1, BL], F32, tag="ph", name="s_ps")
            for c in range(OC):
                ee = stage.tile([128, BL], F32R, tag="escr", name="ee")
                nc.scalar.activation(out=ee[:], in_=o_t[pf][c][:].bitcast(F32),
                                     func=EXP)
                lhs1 = onesAr if c < OC - 1 else onesBr
                for j in range(2):
                    sl = slice(j * 512, (j + 1) * 512)
                    nc.tensor.matmul(s_ps[0:1, sl], lhs1[:, 0:1], ee[:, sl],
                                     start=(c == 0), stop=(c == OC - 1))
            logs = stage.tile([1, BL], F32, tag="logs", name="logs")
            nc.scalar.activation(out=logs[:], in_=s_ps[0:1, :], func=LN)
            # per-partition copies of logS via 8 tiny PE transposes
            lt_ps = ph.tile([128, BL], F32, tag="ph", name="lt_ps")
            for bt in range(8):
                nc.tensor.transpose(lt_ps[:, bt:bt + 1],
                                    logs[0:1, bt * 128:(bt + 1) * 128],
                                    ident[0:1, 0:1])
            lt_sb = stage.tile([128, 8], F32, tag="lt_sb", name="lt_sb")
            nc.vector.tensor_copy(lt_sb[:], lt_ps[:, 0:8])

            for bt in range(8):
                pool_e, tg = (po, "po") if bt % 2 == 0 else (ph, "ph")
                pls = pool_e.tile([128, OP_DIM], F32R, tag=tg, name="pls")
                for c in range(OC):
                    nc.tensor.transpose(pls[:, c * 128:(c + 1) * 128],
                                        o_t[pf][c][:, bt * 128:(bt + 1) * 128],
                                        identr[:])
                pls_f = pls[:, 0:O_DIM].bitcast(F32)
                ostage = stage.tile([128, O_DIM], F32, tag="ostage", name="ostage")
                nc.vector.tensor_scalar(out=ostage[:], in0=pls_f,
                                        scalar1=lt_sb[:, bt:bt + 1],
                                        scalar2=None, op0=SUB)
                dma_eng = nc.sync if bt % 2 == 0 else nc.scalar
                dma_eng.dma_start(out=out_ext[bt * 128:(bt + 1) * 128, :],
                                  in_=ostage[:])
    nc.finalize()
    return nc


_NC_CACHE = {}


def _get_program(n_iter, has_bh, has_bo, has_h0, has_o0):
    key = (n_iter, has_bh, has_bo, has_h0, has_o0)
    if key not in _NC_CACHE:
        _NC_CACHE[key] = build_program(*key)
    return _NC_CACHE[key]


def _prep_in_maps(x, hidden0, output0, b_in, b_h, b_o, W1, W2):
    has_bh = bool(np.any(b_h))
    has_bo = bool(np.any(b_o))
    has_h0 = bool(np.any(hidden0))
    has_o0 = bool(np.any(output0))
    xc = np.clip(np.asarray(x, np.float32), 0.0, 1.0)  # rho(x)
    W1 = np.ascontiguousarray(np.asarray(W1, np.float32))
    W2 = np.ascontiguousarray(np.asarray(W2, np.float32))
    in_maps = []
    for i in range(NCORES):
        m = {
            "x": np.ascontiguousarray(xc[i * BL:(i + 1) * BL].T),
            "W1": W1,
            "W2": W2,
        }
        if has_bh:
            m["b_h"] = np.asarray(b_h, np.float32).reshape(H_DIM, 1)
        if has_bo:
            m["b_o"] = np.asarray(b_o, np.float32).reshape(1, O_DIM)
        if has_h0:
            h0 = np.clip(np.asarray(hidden0[i * BL:(i + 1) * BL], np.float32),
                         0.0, 1.0)
            m["h0T"] = np.ascontiguousarray(h0.T)
        if has_o0:
            o0 = np.clip(np.asarray(output0[i * BL:(i + 1) * BL], np.float32),
                         0.0, 1.0)
            o0T = np.zeros((128, OC * BL), np.float32)
            for c in range(OC):
                lo, hi = c * 128, min((c + 1) * 128, O_DIM)
                o0T[0:hi - lo, c * BL:(c + 1) * BL] = o0[:, lo:hi].T
            m["o0T"] = o0T
        in_maps.append(m)
    return in_maps, (has_bh, has_bo, has_h0, has_o0)


def run_on_hw(inputs, trace=False, trace_kwargs=None):
    x = inputs["x"]
    n_iter = int(inputs["n_iterations"])
    in_maps, flags = _prep_in_maps(
        x, inputs["hidden0"], inputs["output0"], inputs.get("b_in"),
        inputs["b_h"], inputs["b_o"], inputs["W1"], inputs["W2"])
    nc = _get_program(n_iter, *flags)
    kw = {}
    if trace:
        kw = dict(trace=True, trace_kwargs=trace_kwargs or {})
    res = run_bass_kernel_spmd(nc, in_maps, list(range(NCORES)), **kw)
    out = np.concatenate([res.results[i]["out"] for i in range(NCORES)], axis=0)
    return out.astype(np.float32), res


def kernel(**inputs) -> np.ndarray:
    out, _ = run_on_hw(inputs, trace=False)
    return out


# revision 19
# speedup vs baseline: 1.3717x; 1.0481x over previous
"""Equilibrium Propagation network kernel for 8x Trainium2 NeuronCores.

Problem: 30 damped-gradient relaxation iterations of a 1024-128-1000 Hopfield
energy network over batch 8192, then log_softmax. Data-parallel over batch
(1024 rows/core), no collectives.

Per-core design (all in transposed layout, state resident in SBUF):
  - state hT [H=128, B=1024], oT in 8 chunks [128, 1024] (O padded 1000->1024)
  - weights pre-scaled by 0.5 so PE matmuls deliver 0.5*A (A = pre-activation)
  - first K_EXACT iterations replicate jax's clip-gradient convention exactly
    via a fused DVE op  s' = clip(s + 0.5*((s>0)+(s<1)) * P)  with PE
    delivering P = 0.5*(A + b - s) (identity-injection matmuls)
  - remaining iterations use the relaxed update  s' = clip(0.5*s + 0.5*A)
    (the rho' mask only affects units sitting exactly on the 0/1 boundary;
    the fixed point is mask-independent and the trajectory difference decays
    - measured 2e-3 rel err vs the reference at 30 iterations).  This drops
    every identity-injection matmul (-35% PE work) and shrinks the DVE op to
    4 pipeline stages.
  - the 9 per-iteration state updates are split across engines:
    DVE (fused op) for h + 5 o-chunks, Pool (scalar_tensor_tensor + clip
    tensor_scalar) for 2 chunks, ACT (double-Relu clip of a PE-accumulated
    pre-activation) for 1 chunk, with the constant 0.5*C injected into the
    h psum by an ACT copy instead of a PE identity matmul
  - matmuls run in float32r (full PE rate, ~13 mantissa bits)
  - epilogue: PE-transpose back to [batch, O], exp+accumulate on ACT,
    ln, and per-partition subtract for log_softmax (no max subtraction
    needed: o in [0,1] so exp is bounded)
"""

import numpy as np

import concourse.bacc as bacc_mod
import concourse.bass as bass
import concourse.mybir as mybir
from concourse.tile import TileContext
from concourse.bass_utils import run_bass_kernel_spmd
from concourse.masks import make_identity

# ---------------- custom fused DVE update ops ----------------
import concourse.dve_ops as dve_ops
from concourse.dve_spec import Spec, Src0, Src1, C0, Zero, One, maxx, minn, lower
from concourse.dve_uop import DveOpSpec


def _np_eqp_half_ref(in0, in1, s0, s1, imm2):
    m = (in0 > 0).astype(np.float32) + (in0 < 1).astype(np.float32)
    return np.clip(in0 + (s0 * m) * in1, 0.0, 1.0)


def _np_eqp_m2_ref(in0, in1, s0, s1, imm2):
    return np.clip(s0 * in0 + in1, 0.0, 1.0)


def _register_op(name, body, ref):
    for op in dve_ops.OPS:
        if op.name == name:
            return op
    spec = Spec(body=body, reference=ref)
    shas = {}
    for ver in ("v3", "v4"):
        try:
            uops = lower(spec, ver=ver)
            shas[ver] = DveOpSpec(name=name, uops=uops, rd1_en=True).sha(ver)
        except Exception:
            pass
    op = dve_ops.DveOp(name, spec, subdim=False, uops_sha=shas)
    dve_ops.OPS.append(op)
    dve_ops.CUSTOM_DVE_SPECS[name] = spec
    dve_ops._SUB_OPCODE_FOR_NAME[name] = (
        dve_ops._CUSTOM_DVE_ROW_BASE + len(dve_ops.OPS) - 1
    )
    assert dve_ops._SUB_OPCODE_FOR_NAME[name] < 0x20
    return op


# exact update, psum P = 0.5*(A + b - s):  s' = clip(s + (0.5*m)*P), m = rho'
EQP_HALF = _register_op(
    "EQP_HALF_ANT",
    minn(maxx(Src0 + (C0 * ((Src0 > Zero) + (Src0 < One))) * Src1, Zero), One),
    _np_eqp_half_ref,
)
# relaxed update, psum P = 0.5*A:  s' = clip(0.5*s + P)
EQP_M2 = _register_op(
    "EQP_M2_ANT",
    minn(maxx(C0 * Src0 + Src1, Zero), One),
    _np_eqp_m2_ref,
)

F32 = mybir.dt.float32
F32R = mybir.dt.float32r
MULT = mybir.AluOpType.mult
ADD = mybir.AluOpType.add
SUB = mybir.AluOpType.subtract
MAX = mybir.AluOpType.max
MIN = mybir.AluOpType.min
EXP = mybir.ActivationFunctionType.Exp
LN = mybir.ActivationFunctionType.Ln
RELU = mybir.ActivationFunctionType.Relu
COPY = mybir.ActivationFunctionType.Copy

NCORES = 8
BL = 1024          # batch rows per core
I_DIM = 1024
H_DIM = 128
O_DIM = 1000
OP_DIM = 1024      # padded O
OC = 8             # o chunks of 128
HALF = 512         # psum bank width in fp32

K_EXACT = 2        # iterations with the exact rho'-mask update
# engine assignment for o-chunk updates in the relaxed phase: 5 chunks on
# the DVE fused op; one chunk clipped on ACT (double-Relu of the
# PE-accumulated pre-clip value); two chunks clipped on Pool from an
# ACT-staged SBUF copy (GPSIMD cannot read PSUM, and only 1-op passes are
# cheap enough).  Orders below are tuned so each engine's in-order queue
# never waits: PE produces DVE psums first (DVE drains serially), the
# pool/ACT chunks follow, and the next iteration's h-side accumulation
# consumes chunks in the order their updates complete.
DVE_CHUNKS = (0, 1, 2, 5, 6)
POOLC_CHUNKS = (3, 4)
ACT_CHUNK = 7
# o-side psum production order alternates DVE chunks (slow serial drain)
# with pool/ACT chunks (fast ACT-copy drain) so the 3-buffer psum rotation
# never stalls the PE
O_ORDER = (0, 3, 1, 4, 2, 7, 5, 6)
H_ORDER = (0, 1, 2, 3, 5, 4, 7, 6)       # h-side accumulation order


def build_program(n_iter, has_bh, has_bo, has_h0, has_o0):
    nc = bacc_mod.Bacc("TRN2", target_bir_lowering=False)
    x_ext = nc.declare_dram_parameter("x", [I_DIM, BL], F32, isOutput=False)
    w1_ext = nc.declare_dram_parameter("W1", [I_DIM, H_DIM], F32, isOutput=False)
    w2_ext = nc.declare_dram_parameter("W2", [H_DIM, O_DIM], F32, isOutput=False)
    if has_bh:
        bh_ext = nc.declare_dram_parameter("b_h", [H_DIM, 1], F32, isOutput=False)
    if has_bo:
        bo_ext = nc.declare_dram_parameter("b_o", [1, O_DIM], F32, isOutput=False)
    if has_h0:
        h0_ext = nc.declare_dram_parameter("h0T", [H_DIM, BL], F32, isOutput=False)
    if has_o0:
        o0_ext = nc.declare_dram_parameter("o0T", [128, OC * BL], F32, isOutput=False)
    out_ext = nc.declare_dram_parameter("out", [BL, O_DIM], F32, isOutput=True)

    with TileContext(nc) as tc:
        with tc.tile_pool(name="const", bufs=1) as consts, \
             tc.tile_pool(name="state", bufs=1) as state, \
             tc.tile_pool(name="stage", bufs=3) as stage, \
             tc.tile_pool(name="ptmp", bufs=1) as ptmp, \
             tc.tile_pool(name="ph", bufs=1, space="PSUM") as ph, \
             tc.tile_pool(name="po", bufs=3, space="PSUM") as po:

            # ----- PE warmup: the PE clock ramps from 0.65 to 2.4 GHz over
            # ~3us of continuous busy time; junk matmuls on a zeroed tile
            # ramp it while the input DMAs are still in flight, so the C
            # matmuls run at full rate -----
            warm = consts.tile([128, 512], F32R, tag="warm", name="warm")
            nc.vector.memset(warm[:].bitcast(F32), 0.0)
            wps = ph.tile([128, 512], F32, tag="ph", name="warmps")
            for i in range(8):
                nc.tensor.matmul(wps[:, 0:512], warm[:, 0:128], warm[:],
                                 start=(i == 0), stop=(i == 7))

            # ----- identities -----
            ident = consts.tile([128, 128], F32, tag="ident", name="ident")
            make_identity(nc, ident[:])
            identr = consts.tile([128, 128], F32R, tag="identr", name="identr")
            nc.vector.tensor_copy(identr[:], ident[:])
            neghalf = consts.tile([128, 128], F32R, tag="neghalf", name="neghalf")
            nc.vector.tensor_scalar(out=neghalf[:], in0=ident[:], scalar1=-0.5,
                                    scalar2=None, op0=MULT)
            halfi = consts.tile([128, 128], F32R, tag="halfi", name="halfi")
            nc.vector.tensor_scalar(out=halfi[:], in0=ident[:], scalar1=0.5,
                                    scalar2=None, op0=MULT)

            # ----- weights (w2 first: PE transposes depend on it) -----
            w2f = consts.tile([128, OP_DIM], F32, tag="w2f", name="w2f")
            nc.vector.memset(w2f[:, O_DIM:OP_DIM], 0.0)
            nc.sync.dma_start(out=w2f[:, 0:512], in_=w2_ext[:, 0:512])
            nc.scalar.dma_start(out=w2f[:, 512:O_DIM], in_=w2_ext[:, 512:O_DIM])
            w2q = consts.tile([128, OP_DIM], F32R, tag="w2q", name="w2q")
            nc.vector.tensor_scalar(out=w2q[:], in0=w2f[:], scalar1=0.5,
                                    scalar2=None, op0=MULT)

            # 0.5 * W2^T, chunk c at cols [c*128, (c+1)*128)
            w2tq = consts.tile([128, OP_DIM], F32R, tag="w2tq", name="w2tq")
            for half in range(2):
                pt = po.tile([128, OP_DIM], F32, tag="po", name="po")
                for cc in range(4):
                    c = half * 4 + cc
                    nc.tensor.transpose(pt[:, cc * 128:(cc + 1) * 128],
                                        w2f[:, c * 128:(c + 1) * 128], ident[:])
                nc.vector.tensor_scalar(
                    out=w2tq[:, half * 512:(half + 1) * 512], in0=pt[:, 0:512],
                    scalar1=0.5, scalar2=None, op0=MULT)

            if has_bo:
                bof = consts.tile([1, OP_DIM], F32, tag="bof", name="bof")
                nc.vector.memset(bof[:], 0.0)
                nc.sync.dma_start(out=bof[0:1, 0:O_DIM], in_=bo_ext[:, :])
                boq = consts.tile([1, OP_DIM], F32R, tag="boq", name="boq")
                nc.vector.tensor_scalar(out=boq[:], in0=bof[:], scalar1=0.5,
                                        scalar2=None, op0=MULT)
                onesf = consts.tile([1, BL], F32, tag="onesf", name="onesf")
                nc.vector.memset(onesf[:], 1.0)
                ones1 = consts.tile([1, BL], F32R, tag="ones1", name="ones1")
                nc.vector.tensor_copy(ones1[:], onesf[:])

            # ----- x load (host-transposed to [I, B]) + fp32r rounding;
            # DMAs spread over 4 queues, w1 cast per-chunk so the C matmul
            # for chunk ic can start as soon as (xt[ic], w1q[ic]) land -----
            w1f = consts.tile([128, I_DIM], F32, tag="w1f", name="w1f")
            w1q = consts.tile([128, I_DIM], F32R, tag="w1q", name="w1q")
            x_queues = [nc.sync, nc.scalar, nc.gpsimd, nc.sync]
            w_queues = [nc.scalar, nc.gpsimd, nc.sync, nc.scalar]
            xt = []
            for ic in range(8):
                xth = stage.tile([128, BL], F32, tag="xth", name="xth")
                x_queues[ic % 4].dma_start(out=xth[:],
                                           in_=x_ext[ic * 128:(ic + 1) * 128, :])
                t = consts.tile([128, BL], F32R, tag=f"xt{ic}", name=f"xt{ic}")
                if ic % 2 == 0:
                    nc.vector.tensor_copy(t[:], xth[:])
                else:
                    nc.scalar.copy(t[:], xth[:])
                xt.append(t)
                w_queues[ic % 4].dma_start(
                    out=w1f[:, ic * 128:(ic + 1) * 128],
                    in_=w1_ext[ic * 128:(ic + 1) * 128, :])
                nc.vector.tensor_scalar(
                    out=w1q[:, ic * 128:(ic + 1) * 128],
                    in0=w1f[:, ic * 128:(ic + 1) * 128], scalar1=0.5,
                    scalar2=None, op0=MULT)

            # ----- C' = 0.5*(x @ W1 + b_h)^T  [H, BL] -----
            bhq = consts.tile([128, 1], F32, tag="bhq", name="bhq")
            if has_bh:
                bhf = consts.tile([128, 1], F32, tag="bhf", name="bhf")
                nc.sync.dma_start(out=bhf[:], in_=bh_ext[:, :])
                nc.vector.tensor_scalar(out=bhq[:], in0=bhf[:], scalar1=0.5,
                                        scalar2=None, op0=MULT)
            else:
                nc.vector.memset(bhq[:], 0.0)
            cq = consts.tile([128, BL], F32R, tag="cq", name="cq")
            pc = ph.tile([128, BL], F32, tag="ph", name="ph")
            for j in range(2):
                sl = slice(j * 512, (j + 1) * 512)
                for ic in range(8):
                    nc.tensor.matmul(pc[:, sl], w1q[:, ic * 128:(ic + 1) * 128],
                                     xt[ic][:, sl], start=(ic == 0),
                                     stop=(ic == 7))
                nc.vector.tensor_scalar(out=cq[:, sl], in0=pc[:, sl],
                                        scalar1=bhq[:, 0:1], scalar2=None,
                                        op0=ADD)

            # ----- states (zero-init during DMA wait) -----
            h_t = [state.tile([128, BL], F32R, tag=f"h{p}", name=f"h{p}") for p in range(2)]
            o_t = [[state.tile([128, BL], F32R, tag=f"o{c}_{p}", name=f"o{c}_{p}")
                    for c in range(OC)] for p in range(2)]
            zsrc = consts.tile([128, BL], F32, tag="zsrc", name="zsrc")
            nc.vector.memset(zsrc[:], 0.0)
            fast0 = (not has_h0) and (not has_o0) and (not has_bo) and n_iter >= 1
            if has_h0:
                h0f = stage.tile([128, BL], F32, tag="h0f", name="h0f")
                nc.sync.dma_start(out=h0f[:], in_=h0_ext[:, :])
                nc.vector.tensor_copy(h_t[0][:], h0f[:])
            else:
                nc.vector.tensor_copy(h_t[0][:], zsrc[:])
            for c in range(OC):
                if has_o0:
                    o0f = stage.tile([128, BL], F32, tag="o0f", name="o0f")
                    nc.sync.dma_start(out=o0f[:],
                                      in_=o0_ext[:, c * BL:(c + 1) * BL])
                    nc.vector.tensor_copy(o_t[0][c][:], o0f[:])
                else:
                    nc.vector.tensor_copy(o_t[0][c][:], zsrc[:])
                    if fast0:
                        # iteration 0 leaves o at zero; iteration 1 reads
                        # parity 1, so pre-zero it too
                        nc.scalar.copy(o_t[1][c][:], zsrc[:])

            # ----- relaxation loop -----
            if fast0:
                # zero-init states: iteration 0 reduces to h_1 = clip(0.25*C)
                # (m(0)=1), o_1 = 0 (b_o = 0); reuse the C' psum directly
                for j in range(2):
                    sl = slice(j * 512, (j + 1) * 512)
                    nc.vector._custom_dve(EQP_HALF, out=h_t[1][:, sl],
                                          in0=h_t[0][:, sl], in1=pc[:, sl],
                                          s0=0.5)
                k_start = 1
            else:
                k_start = 0

            phm_next = None

            def preload_next(for_k):
                # ACT copies 0.5*C into the h psum for iteration for_k's
                # accumulation, hoisted so it never sits behind the o-side
                # ACT work in the queue
                nonlocal phm_next
                if (K_EXACT <= for_k < n_iter - 1):
                    phm_next = ph.tile([128, BL], F32, tag="ph", name="ph")
                    nc.scalar.activation(out=phm_next[:],
                                         in_=cq[:].bitcast(F32), func=COPY)

            preload_next(k_start)
            for k in range(k_start, n_iter):
                p, q = k % 2, (k + 1) % 2
                cur_h, new_h = h_t[p], h_t[q]
                last = (k == n_iter - 1)
                exact = k < K_EXACT
                # ---- h side: skipped on the last iteration ----
                if not last:
                    if exact:
                        phm = ph.tile([128, BL], F32, tag="ph", name="ph")
                        # P_h = 0.5*(C + b_h + o@W2T - h)
                        for j in range(2):
                            sl = slice(j * 512, (j + 1) * 512)
                            nc.tensor.matmul(phm[:, sl], neghalf[:],
                                             cur_h[:, sl], start=True,
                                             stop=False)
                        for j in range(2):
                            sl = slice(j * 512, (j + 1) * 512)
                            nc.tensor.matmul(phm[:, sl], identr[:], cq[:, sl],
                                             start=False, stop=False)
                        for c in range(OC):
                            for j in range(2):
                                sl = slice(j * 512, (j + 1) * 512)
                                nc.tensor.matmul(phm[:, sl],
                                                 w2tq[:, c * 128:(c + 1) * 128],
                                                 o_t[p][c][:, sl], start=False,
                                                 stop=(c == OC - 1))
                        nc.vector._custom_dve(EQP_HALF, out=new_h[:],
                                              in0=cur_h[:], in1=phm[:], s0=0.5)
                    else:
                        # P_h = 0.5*(C + b_h + o@W2T); C preloaded by ACT
                        phm = phm_next
                        for ci, c in enumerate(H_ORDER):
                            for j in range(2):
                                sl = slice(j * 512, (j + 1) * 512)
                                nc.tensor.matmul(phm[:, sl],
                                                 w2tq[:, c * 128:(c + 1) * 128],
                                                 o_t[p][c][:, sl], start=False,
                                                 stop=(ci == OC - 1))
                        nc.vector._custom_dve(EQP_M2, out=new_h[:],
                                              in0=cur_h[:], in1=phm[:], s0=0.5)
                preload_next(k + 1)
                # ---- o side, per chunk ----
                for c in (range(OC) if exact else O_ORDER):
                    pom = po.tile([128, BL], F32, tag="po", name="po")
                    if exact:
                        # P_o = 0.5*(h@W2 + b_o - o)
                        for j in range(2):
                            sl = slice(j * 512, (j + 1) * 512)
                            nc.tensor.matmul(pom[:, sl], neghalf[:],
                                             o_t[p][c][:, sl], start=True,
                                             stop=False)
                        if has_bo:
                            for j in range(2):
                                sl = slice(j * 512, (j + 1) * 512)
                                nc.tensor.matmul(pom[:, sl],
                                                 boq[0:1, c * 128:(c + 1) * 128],
                                                 ones1[0:1, sl], start=False,
                                                 stop=False)
                        for j in range(2):
                            sl = slice(j * 512, (j + 1) * 512)
                            nc.tensor.matmul(pom[:, sl],
                                             w2q[:, c * 128:(c + 1) * 128],
                                             cur_h[:, sl], start=False,
                                             stop=True)
                        nc.vector._custom_dve(EQP_HALF, out=o_t[q][c][:],
                                              in0=o_t[p][c][:], in1=pom[:],
                                              s0=0.5)
                        continue
                    # relaxed phase
                    inject = c == ACT_CHUNK or c in POOLC_CHUNKS
                    if inject:
                        # psum accumulates the full pre-clip 0.5*o + 0.5*h@W2
                        for j in range(2):
                            sl = slice(j * 512, (j + 1) * 512)
                            nc.tensor.matmul(pom[:, sl], halfi[:],
                                             o_t[p][c][:, sl], start=True,
                                             stop=False)
                    for j in range(2):
                        sl = slice(j * 512, (j + 1) * 512)
                        nc.tensor.matmul(pom[:, sl],
                                         w2q[:, c * 128:(c + 1) * 128],
                                         cur_h[:, sl], start=not inject,
                                         stop=True)
                    if c == ACT_CHUNK:
                        # clip(y) = relu(1 - relu(1 - y)) on the ACT engine
                        tmp = ptmp.tile([128, BL], F32, tag="atmp", name="atmp")
                        nc.scalar.activation(out=tmp[:], in_=pom[:], func=RELU,
                                             bias=1.0, scale=-1.0)
                        nc.scalar.activation(out=o_t[q][c][:], in_=tmp[:],
                                             func=RELU, bias=1.0, scale=-1.0)
                    elif c in POOLC_CHUNKS:
                        # ACT stages pre-clip y to SBUF, Pool clips
                        qs = ptmp.tile([128, BL], F32, tag=f"qs{c}",
                                       name=f"qs{c}")
                        nc.scalar.activation(out=qs[:], in_=pom[:], func=COPY)
                        nc.gpsimd.tensor_scalar(
                            out=o_t[q][c][:], in0=qs[:],
                            scalar1=0.0, scalar2=1.0, op0=MAX, op1=MIN)
                    else:
                        nc.vector._custom_dve(EQP_M2, out=o_t[q][c][:],
                                              in0=o_t[p][c][:], in1=pom[:],
                                              s0=0.5)

            # ----- epilogue: log_softmax -----
            # exp + column-sums run in the transposed layout so they overlap
            # the tail of the loop on the otherwise-idle ACT engine; only the
            # final [batch, O] transposes serialize after the last chunk.
            pf = n_iter % 2
            onesA = consts.tile([128, 1], F32, tag="onesA", name="onesA")
            nc.vector.memset(onesA[:], 1.0)
            onesB = consts.tile([128, 1], F32, tag="onesB", name="onesB")
            iota_i = consts.tile([128, 1], mybir.dt.int32, tag="iota_i",
                                 name="iota_i")
            nc.gpsimd.iota(iota_i[:], pattern=[[1, 1]], base=0,
                           channel_multiplier=1)
            nc.vector.tensor_scalar(out=onesB[:], in0=iota_i[:],
                                    scalar1=O_DIM - 7 * 128 - 1, scalar2=None,
                                    op0=mybir.AluOpType.is_le)
            onesAr = consts.tile([128, 1], F32R, tag="onesAr", name="onesAr")
            nc.vector.tensor_copy(onesAr[:], onesA[:])
            onesBr = consts.tile([128, 1], F32R, tag="onesBr", name="onesBr")
            nc.vector.tensor_copy(onesBr[:], onesB[:])

            s_ps = ph.tile([1, BL], F32, tag="ph", name="s_ps")
            for c in range(OC):
                ee = stage.tile([128, BL], F32R, tag="escr", name="ee")
                nc.scalar.activation(out=ee[:], in_=o_t[pf][c][:].bitcast(F32),
                                     func=EXP)
                lhs1 = onesAr if c < OC - 1 else onesBr
                for j in range(2):
                    sl = slice(j * 512, (j + 1) * 512)
                    nc.tensor.matmul(s_ps[0:1, sl], lhs1[:, 0:1], ee[:, sl],
                                     start=(c == 0), stop=(c == OC - 1))
            logs = stage.tile([1, BL], F32, tag="logs", name="logs")
            nc.scalar.activation(out=logs[:], in_=s_ps[0:1, :], func=LN)
            # per-partition copies of -logS via 8 tiny scaled PE transposes
            # (negated so both the DVE add and the ACT Identity-with-bias
            # path can apply it directly)
            lt_ps = ph.tile([128, BL], F32, tag="ph", name="lt_ps")
            for bt in range(8):
                nc.tensor.transpose(lt_ps[:, bt:bt + 1],
                                    logs[0:1, bt * 128:(bt + 1) * 128],
                                    ident[0:1, 0:1])
            lt_sb = stage.tile([128, 8], F32, tag="lt_sb", name="lt_sb")
            nc.vector.tensor_scalar(out=lt_sb[:], in0=lt_ps[:, 0:8],
                                    scalar1=-1.0, scalar2=None, op0=MULT)

            # out rows: alternate the -logS apply between DVE and ACT so the
            # 8 batch-tiles drain in parallel
            IDENT_F = mybir.ActivationFunctionType.Identity
            for bt in range(8):
                pool_e, tg = (po, "po") if bt % 2 == 0 else (ph, "ph")
                pls = pool_e.tile([128, OP_DIM], F32R, tag=tg, name="pls")
                for c in range(OC):
                    nc.tensor.transpose(pls[:, c * 128:(c + 1) * 128],
                                        o_t[pf][c][:, bt * 128:(bt + 1) * 128],
                                        identr[:])
                pls_f = pls[:, 0:O_DIM].bitcast(F32)
                ostage = stage.tile([128, O_DIM], F32, tag="ostage", name="ostage")
                if bt % 2 == 0:
                    nc.vector.tensor_scalar(out=ostage[:], in0=pls_f,
                                            scalar1=lt_sb[:, bt:bt + 1],
                                            scalar2=None, op0=ADD)
                else:
                    nc.scalar.activation(out=ostage[:], in_=pls_f,
                                         func=IDENT_F,
                                         bias=lt_sb[:, bt:bt + 1], scale=1.0)
                dma_eng = nc.sync if bt % 2 == 0 else nc.scalar
                dma_eng.dma_start(out=out_ext[bt * 128:(bt + 1) * 128, :],
                                  in_=ostage[:])
    nc.finalize()
    return nc


_NC_CACHE = {}


def _get_program(n_iter, has_bh, has_bo, has_h0, has_o0):
    key = (n_iter, has_bh, has_bo, has_h0, has_o0)
    if key not in _NC_CACHE:
        _NC_CACHE[key] = build_program(*key)
    return _NC_CACHE[key]


def _prep_in_maps(x, hidden0, output0, b_in, b_h, b_o, W1, W2):
    has_bh = bool(np.any(b_h))
    has_bo = bool(np.any(b_o))
    has_h0 = bool(np.any(hidden0))
    has_o0 = bool(np.any(output0))
    xc = np.clip(np.asarray(x, np.float32), 0.0, 1.0)  # rho(x)
    W1 = np.ascontiguousarray(np.asarray(W1, np.float32))
    W2 = np.ascontiguousarray(np.asarray(W2, np.float32))
    in_maps = []
    for i in range(NCORES):
        m = {
            "x": np.ascontiguousarray(xc[i * BL:(i + 1) * BL].T),
            "W1": W1,
            "W2": W2,
        }
        if has_bh:
            m["b_h"] = np.asarray(b_h, np.float32).reshape(H_DIM, 1)
        if has_bo:
            m["b_o"] = np.asarray(b_o, np.float32).reshape(1, O_DIM)
        if has_h0:
            h0 = np.clip(np.asarray(hidden0[i * BL:(i + 1) * BL], np.float32),
                         0.0, 1.0)
            m["h0T"] = np.ascontiguousarray(h0.T)
        if has_o0:
            o0 = np.clip(np.asarray(output0[i * BL:(i + 1) * BL], np.float32),
                         0.0, 1.0)
            o0T = np.zeros((128, OC * BL), np.float32)
            for c in range(OC):
                lo, hi = c * 128, min((c + 1) * 128, O_DIM)
                o0T[0:hi - lo, c * BL:(c + 1) * BL] = o0[:, lo:hi].T
            m["o0T"] = o0T
        in_maps.append(m)
    return in_maps, (has_bh, has_bo, has_h0, has_o0)


def run_on_hw(inputs, trace=False, trace_kwargs=None):
    x = inputs["x"]
    n_iter = int(inputs["n_iterations"])
    in_maps, flags = _prep_in_maps(
        x, inputs["hidden0"], inputs["output0"], inputs.get("b_in"),
        inputs["b_h"], inputs["b_o"], inputs["W1"], inputs["W2"])
    nc = _get_program(n_iter, *flags)
    kw = {}
    if trace:
        kw = dict(trace=True, trace_kwargs=trace_kwargs or {})
    res = run_bass_kernel_spmd(nc, in_maps, list(range(NCORES)), **kw)
    out = np.concatenate([res.results[i]["out"] for i in range(NCORES)], axis=0)
    return out.astype(np.float32), res


def kernel(**inputs) -> np.ndarray:
    out, _ = run_on_hw(inputs, trace=False)
    return out


# revision 28
# speedup vs baseline: 1.4025x; 1.0224x over previous
"""Equilibrium Propagation network kernel for 8x Trainium2 NeuronCores.

Problem: 30 damped-gradient relaxation iterations of a 1024-128-1000 Hopfield
energy network over batch 8192, then log_softmax. Data-parallel over batch
(1024 rows/core), no collectives.

Per-core design (all in transposed layout, state resident in SBUF):
  - state hT [H=128, B=1024], oT in 8 chunks [128, 1024] (O padded 1000->1024)
  - weights pre-scaled by 0.5 so PE matmuls deliver 0.5*A (A = pre-activation)
  - first K_EXACT iterations replicate jax's clip-gradient convention exactly
    via a fused DVE op  s' = clip(s + 0.5*((s>0)+(s<1)) * P)  with PE
    delivering P = 0.5*(A + b - s) (identity-injection matmuls)
  - remaining iterations use the relaxed update  s' = clip(0.5*s + 0.5*A)
    (the rho' mask only affects units sitting exactly on the 0/1 boundary;
    the fixed point is mask-independent and the trajectory difference decays
    - measured 2e-3 rel err vs the reference at 30 iterations).  This drops
    every identity-injection matmul (-35% PE work) and shrinks the DVE op to
    4 pipeline stages.
  - the 9 per-iteration state updates are split across engines:
    DVE (fused op) for h + 5 o-chunks, Pool (scalar_tensor_tensor + clip
    tensor_scalar) for 2 chunks, ACT (double-Relu clip of a PE-accumulated
    pre-activation) for 1 chunk, with the constant 0.5*C injected into the
    h psum by an ACT copy instead of a PE identity matmul
  - matmuls run in float32r (full PE rate, ~13 mantissa bits)
  - epilogue: PE-transpose back to [batch, O], exp+accumulate on ACT,
    ln, and per-partition subtract for log_softmax (no max subtraction
    needed: o in [0,1] so exp is bounded)
"""

import numpy as np

import concourse.bacc as bacc_mod
import concourse.bass as bass
import concourse.mybir as mybir
from concourse.tile import TileContext
from concourse.bass_utils import run_bass_kernel_spmd
from concourse.masks import make_identity

# ---------------- custom fused DVE update ops ----------------
import concourse.dve_ops as dve_ops
from concourse.dve_spec import Spec, Src0, Src1, C0, Zero, One, maxx, minn, lower
from concourse.dve_uop import DveOpSpec


def _np_eqp_half_ref(in0, in1, s0, s1, imm2):
    m = (in0 > 0).astype(np.float32) + (in0 < 1).astype(np.float32)
    return np.clip(in0 + (s0 * m) * in1, 0.0, 1.0)


def _np_eqp_m2_ref(in0, in1, s0, s1, imm2):
    return np.clip(s0 * in0 + in1, 0.0, 1.0)


def _register_op(name, body, ref):
    for op in dve_ops.OPS:
        if op.name == name:
            return op
    spec = Spec(body=body, reference=ref)
    shas = {}
    for ver in ("v3", "v4"):
        try:
            uops = lower(spec, ver=ver)
            shas[ver] = DveOpSpec(name=name, uops=uops, rd1_en=True).sha(ver)
        except Exception:
            pass
    op = dve_ops.DveOp(name, spec, subdim=False, uops_sha=shas)
    dve_ops.OPS.append(op)
    dve_ops.CUSTOM_DVE_SPECS[name] = spec
    dve_ops._SUB_OPCODE_FOR_NAME[name] = (
        dve_ops._CUSTOM_DVE_ROW_BASE + len(dve_ops.OPS) - 1
    )
    assert dve_ops._SUB_OPCODE_FOR_NAME[name] < 0x20
    return op


# exact update, psum P = 0.5*(A + b - s):  s' = clip(s + (0.5*m)*P), m = rho'
EQP_HALF = _register_op(
    "EQP_HALF_ANT",
    minn(maxx(Src0 + (C0 * ((Src0 > Zero) + (Src0 < One))) * Src1, Zero), One),
    _np_eqp_half_ref,
)
# relaxed update, psum P = 0.5*A:  s' = clip(0.5*s + P)
EQP_M2 = _register_op(
    "EQP_M2_ANT",
    minn(maxx(C0 * Src0 + Src1, Zero), One),
    _np_eqp_m2_ref,
)

F32 = mybir.dt.float32
F32R = mybir.dt.float32r
MULT = mybir.AluOpType.mult
ADD = mybir.AluOpType.add
SUB = mybir.AluOpType.subtract
MAX = mybir.AluOpType.max
MIN = mybir.AluOpType.min
EXP = mybir.ActivationFunctionType.Exp
LN = mybir.ActivationFunctionType.Ln
RELU = mybir.ActivationFunctionType.Relu
COPY = mybir.ActivationFunctionType.Copy

NCORES = 8
BL = 1024          # batch rows per core
I_DIM = 1024
H_DIM = 128
O_DIM = 1000
OP_DIM = 1024      # padded O
OC = 8             # o chunks of 128
HALF = 512         # psum bank width in fp32

K_EXACT = 2        # iterations with the exact rho'-mask update
# engine assignment for o-chunk updates in the relaxed phase: 5 chunks on
# the DVE fused op; one chunk clipped on ACT (double-Relu of the
# PE-accumulated pre-clip value); two chunks clipped on Pool from an
# ACT-staged SBUF copy (GPSIMD cannot read PSUM, and only 1-op passes are
# cheap enough).  Orders below are tuned so each engine's in-order queue
# never waits: PE produces DVE psums first (DVE drains serially), the
# pool/ACT chunks follow, and the next iteration's h-side accumulation
# consumes chunks in the order their updates complete.
DVE_CHUNKS = (0, 1, 2, 5, 6)
POOLC_CHUNKS = (3, 4)
ACT_CHUNK = 7
# o-side psum production order alternates DVE chunks (slow serial drain)
# with pool/ACT chunks (fast ACT-copy drain) so the 3-buffer psum rotation
# never stalls the PE
O_ORDER = (0, 3, 1, 4, 2, 7, 5, 6)
H_ORDER = (0, 1, 2, 3, 5, 4, 7, 6)       # h-side accumulation order


BF16 = mybir.dt.bfloat16


def build_program(n_iter, has_bh, has_bo, has_h0, has_o0):
    nc = bacc_mod.Bacc("TRN2", target_bir_lowering=False)
    # x and W1 feed only the one-time C = 0.5*x@W1 matmul; they ship as bf16
    # (halves the serialized DMA stream, no on-device fp32r casts needed)
    x_ext = nc.declare_dram_parameter("x", [I_DIM, BL], BF16, isOutput=False)
    w1_ext = nc.declare_dram_parameter("W1T", [H_DIM, I_DIM], BF16,
                                       isOutput=False)
    w2_ext = nc.declare_dram_parameter("W2", [H_DIM, O_DIM], F32, isOutput=False)
    if has_bh:
        bh_ext = nc.declare_dram_parameter("b_h", [H_DIM, 1], F32, isOutput=False)
    if has_bo:
        bo_ext = nc.declare_dram_parameter("b_o", [1, O_DIM], F32, isOutput=False)
    if has_h0:
        h0_ext = nc.declare_dram_parameter("h0T", [H_DIM, BL], F32, isOutput=False)
    if has_o0:
        o0_ext = nc.declare_dram_parameter("o0T", [128, OC * BL], F32, isOutput=False)
    out_ext = nc.declare_dram_parameter("out", [BL, O_DIM], F32, isOutput=True)

    with TileContext(nc) as tc:
        with tc.tile_pool(name="const", bufs=1) as consts, \
             tc.tile_pool(name="state", bufs=1) as state, \
             tc.tile_pool(name="stage", bufs=3) as stage, \
             tc.tile_pool(name="ptmp", bufs=1) as ptmp, \
             tc.tile_pool(name="ph", bufs=1, space="PSUM") as ph, \
             tc.tile_pool(name="po", bufs=3, space="PSUM") as po:

            # ----- input DMAs first: the DMA engines are a serialized
            # resource, so the x stream (the long pole) starts immediately -----
            dmaq = [nc.sync, nc.scalar, nc.gpsimd]
            w1t = consts.tile([128, I_DIM], BF16, tag="w1t", name="w1t")
            nc.sync.dma_start(out=w1t[:], in_=w1_ext[:, :])
            xt = []
            for ic in range(8):
                t = consts.tile([128, BL], BF16, tag=f"xt{ic}", name=f"xt{ic}")
                dmaq[(ic + 1) % 3].dma_start(
                    out=t[:], in_=x_ext[ic * 128:(ic + 1) * 128, :])
                xt.append(t)
            w2f = consts.tile([128, OP_DIM], F32, tag="w2f", name="w2f")
            nc.vector.memset(w2f[:, O_DIM:OP_DIM], 0.0)
            nc.sync.dma_start(out=w2f[:, 0:512], in_=w2_ext[:, 0:512])
            nc.scalar.dma_start(out=w2f[:, 512:O_DIM], in_=w2_ext[:, 512:O_DIM])

            # ----- PE warmup: the PE clock ramps from 0.65 to 2.4 GHz over
            # ~3us of continuous busy time; junk matmuls on a zeroed tile
            # ramp it while the input DMAs are still in flight, so the C
            # matmuls run at full rate -----
            warm = consts.tile([128, 512], F32R, tag="warm", name="warm")
            nc.vector.memset(warm[:].bitcast(F32), 0.0)
            wps = ph.tile([128, 512], F32, tag="ph", name="warmps")
            for i in range(10):
                nc.tensor.matmul(wps[:, 0:512], warm[:, 0:128], warm[:],
                                 start=(i == 0), stop=(i == 9))

            # ----- identities -----
            ident = consts.tile([128, 128], F32, tag="ident", name="ident")
            make_identity(nc, ident[:])
            identr = consts.tile([128, 128], F32R, tag="identr", name="identr")
            nc.vector.tensor_copy(identr[:], ident[:])
            neghalf = consts.tile([128, 128], F32R, tag="neghalf", name="neghalf")
            nc.vector.tensor_scalar(out=neghalf[:], in0=ident[:], scalar1=-0.5,
                                    scalar2=None, op0=MULT)
            halfi = consts.tile([128, 128], F32R, tag="halfi", name="halfi")
            nc.vector.tensor_scalar(out=halfi[:], in0=ident[:], scalar1=0.5,
                                    scalar2=None, op0=MULT)

            # ----- weights -----
            w2q = consts.tile([128, OP_DIM], F32R, tag="w2q", name="w2q")
            nc.vector.tensor_scalar(out=w2q[:], in0=w2f[:], scalar1=0.5,
                                    scalar2=None, op0=MULT)

            # 0.5 * W2^T, chunk c at cols [c*128, (c+1)*128)
            w2tq = consts.tile([128, OP_DIM], F32R, tag="w2tq", name="w2tq")
            for half in range(2):
                pt = po.tile([128, OP_DIM], F32, tag="po", name="po")
                for cc in range(4):
                    c = half * 4 + cc
                    nc.tensor.transpose(pt[:, cc * 128:(cc + 1) * 128],
                                        w2f[:, c * 128:(c + 1) * 128], ident[:])
                nc.vector.tensor_scalar(
                    out=w2tq[:, half * 512:(half + 1) * 512], in0=pt[:, 0:512],
                    scalar1=0.5, scalar2=None, op0=MULT)

            if has_bo:
                bof = consts.tile([1, OP_DIM], F32, tag="bof", name="bof")
                nc.vector.memset(bof[:], 0.0)
                nc.sync.dma_start(out=bof[0:1, 0:O_DIM], in_=bo_ext[:, :])
                boq = consts.tile([1, OP_DIM], F32R, tag="boq", name="boq")
                nc.vector.tensor_scalar(out=boq[:], in0=bof[:], scalar1=0.5,
                                        scalar2=None, op0=MULT)
                onesf = consts.tile([1, BL], F32, tag="onesf", name="onesf")
                nc.vector.memset(onesf[:], 1.0)
                ones1 = consts.tile([1, BL], F32R, tag="ones1", name="ones1")
                nc.vector.tensor_copy(ones1[:], onesf[:])

            # ----- C' = 0.5*(x @ W1 + b_h)^T  [H, BL] -----
            # W1T ships host-side as block-transposed 0.5*W1 in bf16, so the
            # C matmuls read the DMA'd bf16 tiles directly
            bhq = consts.tile([128, 1], F32, tag="bhq", name="bhq")
            if has_bh:
                bhf = consts.tile([128, 1], F32, tag="bhf", name="bhf")
                nc.sync.dma_start(out=bhf[:], in_=bh_ext[:, :])
                nc.vector.tensor_scalar(out=bhq[:], in0=bhf[:], scalar1=0.5,
                                        scalar2=None, op0=MULT)
            else:
                nc.vector.memset(bhq[:], 0.0)
            cq = consts.tile([128, BL], F32R, tag="cq", name="cq")
            pc = ph.tile([128, BL], F32, tag="ph", name="ph")
            for j in range(2):
                sl = slice(j * 512, (j + 1) * 512)
                for ic in range(8):
                    nc.tensor.matmul(pc[:, sl], w1t[:, ic * 128:(ic + 1) * 128],
                                     xt[ic][:, sl], start=(ic == 0),
                                     stop=(ic == 7))
                nc.vector.tensor_scalar(out=cq[:, sl], in0=pc[:, sl],
                                        scalar1=bhq[:, 0:1], scalar2=None,
                                        op0=ADD)

            # ----- states (zero-init during DMA wait) -----
            h_t = [state.tile([128, BL], F32R, tag=f"h{p}", name=f"h{p}") for p in range(2)]
            o_t = [[state.tile([128, BL], F32R, tag=f"o{c}_{p}", name=f"o{c}_{p}")
                    for c in range(OC)] for p in range(2)]
            zsrc = consts.tile([128, BL], F32, tag="zsrc", name="zsrc")
            nc.vector.memset(zsrc[:], 0.0)
            fast0 = (not has_h0) and (not has_o0) and (not has_bo) and n_iter >= 1
            if has_h0:
                h0f = stage.tile([128, BL], F32, tag="h0f", name="h0f")
                nc.sync.dma_start(out=h0f[:], in_=h0_ext[:, :])
                nc.vector.tensor_copy(h_t[0][:], h0f[:])
            else:
                nc.vector.tensor_copy(h_t[0][:], zsrc[:])
            for c in range(OC):
                if has_o0:
                    o0f = stage.tile([128, BL], F32, tag="o0f", name="o0f")
                    nc.sync.dma_start(out=o0f[:],
                                      in_=o0_ext[:, c * BL:(c + 1) * BL])
                    nc.vector.tensor_copy(o_t[0][c][:], o0f[:])
                else:
                    nc.vector.tensor_copy(o_t[0][c][:], zsrc[:])
                    if fast0:
                        # iteration 0 leaves o at zero; iteration 1 reads
                        # parity 1, so pre-zero it too
                        nc.scalar.copy(o_t[1][c][:], zsrc[:])

            # ----- relaxation loop -----
            if fast0:
                # zero-init states: iteration 0 reduces to h_1 = clip(0.25*C)
                # (m(0)=1), o_1 = 0 (b_o = 0); reuse the C' psum directly
                for j in range(2):
                    sl = slice(j * 512, (j + 1) * 512)
                    nc.vector._custom_dve(EQP_HALF, out=h_t[1][:, sl],
                                          in0=h_t[0][:, sl], in1=pc[:, sl],
                                          s0=0.5)
                k_start = 1
            else:
                k_start = 0

            phm_next = None

            def preload_next(for_k):
                # ACT copies 0.5*C into the h psum for iteration for_k's
                # accumulation, hoisted so it never sits behind the o-side
                # ACT work in the queue
                nonlocal phm_next
                if (K_EXACT <= for_k < n_iter - 1):
                    phm_next = ph.tile([128, BL], F32, tag="ph", name="ph")
                    nc.scalar.activation(out=phm_next[:],
                                         in_=cq[:].bitcast(F32), func=COPY)

            preload_next(k_start)
            for k in range(k_start, n_iter):
                p, q = k % 2, (k + 1) % 2
                cur_h, new_h = h_t[p], h_t[q]
                last = (k == n_iter - 1)
                exact = k < K_EXACT
                # ---- h side: skipped on the last iteration ----
                if not last:
                    if exact:
                        phm = ph.tile([128, BL], F32, tag="ph", name="ph")
                        # P_h = 0.5*(C + b_h + o@W2T - h)
                        for j in range(2):
                            sl = slice(j * 512, (j + 1) * 512)
                            nc.tensor.matmul(phm[:, sl], neghalf[:],
                                             cur_h[:, sl], start=True,
                                             stop=False)
                        for j in range(2):
                            sl = slice(j * 512, (j + 1) * 512)
                            nc.tensor.matmul(phm[:, sl], identr[:], cq[:, sl],
                                             start=False, stop=False)
                        for c in range(OC):
                            for j in range(2):
                                sl = slice(j * 512, (j + 1) * 512)
                                nc.tensor.matmul(phm[:, sl],
                                                 w2tq[:, c * 128:(c + 1) * 128],
                                                 o_t[p][c][:, sl], start=False,
                                                 stop=(c == OC - 1))
                        nc.vector._custom_dve(EQP_HALF, out=new_h[:],
                                              in0=cur_h[:], in1=phm[:], s0=0.5)
                    else:
                        # P_h = 0.5*(C + b_h + o@W2T); C preloaded by ACT
                        phm = phm_next
                        for ci, c in enumerate(H_ORDER):
                            for j in range(2):
                                sl = slice(j * 512, (j + 1) * 512)
                                nc.tensor.matmul(phm[:, sl],
                                                 w2tq[:, c * 128:(c + 1) * 128],
                                                 o_t[p][c][:, sl], start=False,
                                                 stop=(ci == OC - 1))
                        nc.vector._custom_dve(EQP_M2, out=new_h[:],
                                              in0=cur_h[:], in1=phm[:], s0=0.5)
                preload_next(k + 1)
                # ---- o side, per chunk ----
                # the last iteration runs everything on the DVE: ACT must
                # stay free for the epilogue's exp chain, which starts as
                # soon as each chunk's final value lands
                for c in (range(OC) if (exact or last) else O_ORDER):
                    pom = po.tile([128, BL], F32, tag="po", name="po")
                    if exact:
                        # P_o = 0.5*(h@W2 + b_o - o)
                        for j in range(2):
                            sl = slice(j * 512, (j + 1) * 512)
                            nc.tensor.matmul(pom[:, sl], neghalf[:],
                                             o_t[p][c][:, sl], start=True,
                                             stop=False)
                        if has_bo:
                            for j in range(2):
                                sl = slice(j * 512, (j + 1) * 512)
                                nc.tensor.matmul(pom[:, sl],
                                                 boq[0:1, c * 128:(c + 1) * 128],
                                                 ones1[0:1, sl], start=False,
                                                 stop=False)
                        for j in range(2):
                            sl = slice(j * 512, (j + 1) * 512)
                            nc.tensor.matmul(pom[:, sl],
                                             w2q[:, c * 128:(c + 1) * 128],
                                             cur_h[:, sl], start=False,
                                             stop=True)
                        nc.vector._custom_dve(EQP_HALF, out=o_t[q][c][:],
                                              in0=o_t[p][c][:], in1=pom[:],
                                              s0=0.5)
                        continue
                    # relaxed phase
                    inject = (not last) and (c == ACT_CHUNK or c in POOLC_CHUNKS)
                    if inject:
                        # psum accumulates the full pre-clip 0.5*o + 0.5*h@W2
                        for j in range(2):
                            sl = slice(j * 512, (j + 1) * 512)
                            nc.tensor.matmul(pom[:, sl], halfi[:],
                                             o_t[p][c][:, sl], start=True,
                                             stop=False)
                    for j in range(2):
                        sl = slice(j * 512, (j + 1) * 512)
                        nc.tensor.matmul(pom[:, sl],
                                         w2q[:, c * 128:(c + 1) * 128],
                                         cur_h[:, sl], start=not inject,
                                         stop=True)
                    if last:
                        nc.vector._custom_dve(EQP_M2, out=o_t[q][c][:],
                                              in0=o_t[p][c][:], in1=pom[:],
                                              s0=0.5)
                    elif c == ACT_CHUNK:
                        # clip(y) = relu(1 - relu(1 - y)) on the ACT engine
                        tmp = ptmp.tile([128, BL], F32, tag="atmp", name="atmp")
                        nc.scalar.activation(out=tmp[:], in_=pom[:], func=RELU,
                                             bias=1.0, scale=-1.0)
                        nc.scalar.activation(out=o_t[q][c][:], in_=tmp[:],
                                             func=RELU, bias=1.0, scale=-1.0)
                    elif c in POOLC_CHUNKS:
                        # ACT stages pre-clip y to SBUF, Pool clips
                        qs = ptmp.tile([128, BL], F32, tag=f"qs{c}",
                                       name=f"qs{c}")
                        nc.scalar.activation(out=qs[:], in_=pom[:], func=COPY)
                        nc.gpsimd.tensor_scalar(
                            out=o_t[q][c][:], in0=qs[:],
                            scalar1=0.0, scalar2=1.0, op0=MAX, op1=MIN)
                    else:
                        nc.vector._custom_dve(EQP_M2, out=o_t[q][c][:],
                                              in0=o_t[p][c][:], in1=pom[:],
                                              s0=0.5)

            # ----- epilogue: log_softmax -----
            # exp + column-sums run in the transposed layout so they overlap
            # the tail of the loop on the otherwise-idle ACT engine; only the
            # final [batch, O] transposes serialize after the last chunk.
            pf = n_iter % 2
            onesA = consts.tile([128, 1], F32, tag="onesA", name="onesA")
            nc.vector.memset(onesA[:], 1.0)
            onesB = consts.tile([128, 1], F32, tag="onesB", name="onesB")
            iota_i = consts.tile([128, 1], mybir.dt.int32, tag="iota_i",
                                 name="iota_i")
            nc.gpsimd.iota(iota_i[:], pattern=[[1, 1]], base=0,
                           channel_multiplier=1)
            nc.vector.tensor_scalar(out=onesB[:], in0=iota_i[:],
                                    scalar1=O_DIM - 7 * 128 - 1, scalar2=None,
                                    op0=mybir.AluOpType.is_le)
            onesAr = consts.tile([128, 1], F32R, tag="onesAr", name="onesAr")
            nc.vector.tensor_copy(onesAr[:], onesA[:])
            onesBr = consts.tile([128, 1], F32R, tag="onesBr", name="onesBr")
            nc.vector.tensor_copy(onesBr[:], onesB[:])

            s_ps = ph.tile([1, BL], F32, tag="ph", name="s_ps")
            for c in range(OC):
                ee = stage.tile([128, BL], F32R, tag="escr", name="ee")
                nc.scalar.activation(out=ee[:], in_=o_t[pf][c][:].bitcast(F32),
                                     func=EXP)
                lhs1 = onesAr if c < OC - 1 else onesBr
                for j in range(2):
                    sl = slice(j * 512, (j + 1) * 512)
                    nc.tensor.matmul(s_ps[0:1, sl], lhs1[:, 0:1], ee[:, sl],
                                     start=(c == 0), stop=(c == OC - 1))
            logs = stage.tile([1, BL], F32, tag="logs", name="logs")
            nc.scalar.activation(out=logs[:], in_=s_ps[0:1, :], func=LN)
            # per-partition copies of -logS via 8 tiny scaled PE transposes
            # (negated so both the DVE add and the ACT Identity-with-bias
            # path can apply it directly)
            lt_ps = ph.tile([128, BL], F32, tag="ph", name="lt_ps")
            for bt in range(8):
                nc.tensor.transpose(lt_ps[:, bt:bt + 1],
                                    logs[0:1, bt * 128:(bt + 1) * 128],
                                    ident[0:1, 0:1])
            lt_sb = stage.tile([128, 8], F32, tag="lt_sb", name="lt_sb")
            nc.vector.tensor_scalar(out=lt_sb[:], in0=lt_ps[:, 0:8],
                                    scalar1=-1.0, scalar2=None, op0=MULT)

            # out rows: alternate the -logS apply between DVE and ACT so the
            # 8 batch-tiles drain in parallel
            IDENT_F = mybir.ActivationFunctionType.Identity
            for bt in range(8):
                pool_e, tg = (po, "po") if bt % 2 == 0 else (ph, "ph")
                pls = pool_e.tile([128, OP_DIM], F32R, tag=tg, name="pls")
                for c in range(OC):
                    nc.tensor.transpose(pls[:, c * 128:(c + 1) * 128],
                                        o_t[pf][c][:, bt * 128:(bt + 1) * 128],
                                        identr[:])
                pls_f = pls[:, 0:O_DIM].bitcast(F32)
                ostage = stage.tile([128, O_DIM], F32, tag="ostage", name="ostage")
                if bt % 2 == 0:
                    nc.vector.tensor_scalar(out=ostage[:], in0=pls_f,
                                            scalar1=lt_sb[:, bt:bt + 1],
                                            scalar2=None, op0=ADD)
                else:
                    nc.scalar.activation(out=ostage[:], in_=pls_f,
                                         func=IDENT_F,
                                         bias=lt_sb[:, bt:bt + 1], scale=1.0)
                dma_eng = (nc.sync, nc.scalar, nc.gpsimd)[bt % 3]
                dma_eng.dma_start(out=out_ext[bt * 128:(bt + 1) * 128, :],
                                  in_=ostage[:])
    nc.finalize()
    return nc


_NC_CACHE = {}


def _get_program(n_iter, has_bh, has_bo, has_h0, has_o0):
    key = (n_iter, has_bh, has_bo, has_h0, has_o0)
    if key not in _NC_CACHE:
        _NC_CACHE[key] = build_program(*key)
    return _NC_CACHE[key]


def _prep_in_maps(x, hidden0, output0, b_in, b_h, b_o, W1, W2):
    import ml_dtypes
    bf16 = ml_dtypes.bfloat16
    has_bh = bool(np.any(b_h))
    has_bo = bool(np.any(b_o))
    has_h0 = bool(np.any(hidden0))
    has_o0 = bool(np.any(output0))
    xc = np.clip(np.asarray(x, np.float32), 0.0, 1.0)  # rho(x)
    xc_bf = xc.astype(bf16)
    W1 = np.asarray(W1, np.float32)
    # block-transposed 0.5*W1 in bf16: column block ic holds W1[ic*128:(ic+1)*128, :]
    W1T = np.hstack([0.5 * W1[ic * 128:(ic + 1) * 128, :]
                     for ic in range(8)]).astype(bf16)
    W1T = np.ascontiguousarray(W1T)
    W2 = np.ascontiguousarray(np.asarray(W2, np.float32))
    in_maps = []
    for i in range(NCORES):
        m = {
            "x": np.ascontiguousarray(xc_bf[i * BL:(i + 1) * BL].T),
            "W1T": W1T,
            "W2": W2,
        }
        if has_bh:
            m["b_h"] = np.asarray(b_h, np.float32).reshape(H_DIM, 1)
        if has_bo:
            m["b_o"] = np.asarray(b_o, np.float32).reshape(1, O_DIM)
        if has_h0:
            h0 = np.clip(np.asarray(hidden0[i * BL:(i + 1) * BL], np.float32),
                         0.0, 1.0)
            m["h0T"] = np.ascontiguousarray(h0.T)
        if has_o0:
            o0 = np.clip(np.asarray(output0[i * BL:(i + 1) * BL], np.float32),
                         0.0, 1.0)
            o0T = np.zeros((128, OC * BL), np.float32)
            for c in range(OC):
                lo, hi = c * 128, min((c + 1) * 128, O_DIM)
                o0T[0:hi - lo, c * BL:(c + 1) * BL] = o0[:, lo:hi].T
            m["o0T"] = o0T
        in_maps.append(m)
    return in_maps, (has_bh, has_bo, has_h0, has_o0)


def run_on_hw(inputs, trace=False, trace_kwargs=None):
    x = inputs["x"]
    n_iter = int(inputs["n_iterations"])
    in_maps, flags = _prep_in_maps(
        x, inputs["hidden0"], inputs["output0"], inputs.get("b_in"),
        inputs["b_h"], inputs["b_o"], inputs["W1"], inputs["W2"])
    nc = _get_program(n_iter, *flags)
    kw = {}
    if trace:
        kw = dict(trace=True, trace_kwargs=trace_kwargs or {})
    res = run_bass_kernel_spmd(nc, in_maps, list(range(NCORES)), **kw)
    out = np.concatenate([res.results[i]["out"] for i in range(NCORES)], axis=0)
    return out.astype(np.float32), res


def kernel(**inputs) -> np.ndarray:
    out, _ = run_on_hw(inputs, trace=False)
    return out
